# revision 46
# baseline (speedup 1.0000x reference)
"""Trainium2 Bass kernel for nn_CrossFusionMamba (2-layer Mamba stack + fusion head).

Self-contained: hardcodes all shapes/sharding. Data-parallel over batch across
8 NeuronCores (8 batch elements per core).

Layout: channels on SBUF partitions, flattened (batch, time) on the free dim
(bt = b*512 + t -> 4096 columns per core). Full-BT tiles everywhere.

Engine assignment for the selective scan (the bottleneck):
  ACT   : dA = exp(A[d,n] * dt)            (per-partition scale)
  DVE   : dBu = (dt*u) . B_n ; h = tensor_tensor_scan(dA, dBu) ; hc = h . C_n
  PE    : y = diag(D) @ u + sum_n I @ hc_n (PSUM accumulation, skip term first)
GpSimd does NO compute: running Pool tensor_tensor concurrently with DVE
tensor_tensor degrades BOTH ~2-4x (SBUF port contention) — measured, twice.
DVE tensor_tensor in packed bf16 SBUF hits the 2x mode (~0.56 ns/col); the
scan runs at ~2.1 ns/col + ~2us fixed, which is the hard floor here.
Batch independence inside one scan op is enforced by poisoning dt at each
batch's first column (dt=1e9 -> dA=exp(-big)=0 -> exact state reset).

B/C rows are spilled to DRAM once per layer and partition-broadcast to
[128, BT] tiles per (d, n) (SP queue for B, gpsimd SWDGE for C).
Weights are loaded via gpsimd casting DMAs (f32->bf16) and PE transposes.
Big transients share one 9-deep rotating slot tag ("W"); slot-reuse WAR
edges are safe because every W tile's readers are emitted within 9
subsequent W allocations.
"""
import sys

if "/opt/trn_rl_repo" not in sys.path:
    sys.path.insert(0, "/opt/trn_rl_repo")

from contextlib import ExitStack

import numpy as np

import concourse.bacc as bacc
import concourse.tile as tile
import concourse.mybir as mybir
from concourse.bass_utils import run_bass_kernel_spmd

f32 = mybir.dt.float32
bf16 = mybir.dt.bfloat16
AF = mybir.ActivationFunctionType
ALU = mybir.AluOpType
AX = mybir.AxisListType

# model dims
B, L, VD, ID = 64, 512, 64, 32
H, DI, DS, DC, DR, NL = 256, 512, 16, 4, 16, 2
NCORES = 8
BS = B // NCORES          # batches per core
BT = BS * L               # free columns per core (4096)
LP = L + DC - 1           # padded per-batch length for conv (515)
PBT = BS * LP             # 4120
HB = H // 128             # 2
DB = DI // 128            # 4
POISON = 1.0e9


WEIGHT_NAMES = [
    "vent_in_w", "vent_in_b", "vent_ln_w", "vent_ln_b",
    "m_in_w", "m_conv_w", "m_conv_b", "m_xproj_w", "m_dt_w", "m_dt_b",
    "m_Alog", "m_D", "m_out_w", "m_ln_w", "m_ln_b",
    "pool_w", "pool_b", "img_w1", "img_b1", "img_w2", "img_b2",
    "head_w1", "head_b1", "head_w2", "head_b2",
]


def _build():
    nc = bacc.Bacc("TRN2", target_bir_lowering=False, debug=False)

    # ---- DRAM I/O ----
    xv_d = nc.dram_tensor("xv", [BS, L, VD], f32, kind="ExternalInput")
    xi_d = nc.dram_tensor("xi", [BS, ID], f32, kind="ExternalInput")
    wd = {}
    for name, shape in [
        ("vent_in_w", [H, VD]), ("vent_in_b", [H]), ("vent_ln_w", [H]), ("vent_ln_b", [H]),
        ("m_in_w", [NL, 2 * DI, H]), ("m_conv_w", [NL, DI, DC]), ("m_conv_b", [NL, DI]),
        ("m_xproj_w", [NL, DR + 2 * DS, DI]), ("m_dt_w", [NL, DI, DR]), ("m_dt_b", [NL, DI]),
        ("m_Alog", [NL, DI, DS]), ("m_D", [NL, DI]), ("m_out_w", [NL, H, DI]),
        ("m_ln_w", [NL, H]), ("m_ln_b", [NL, H]),
        ("pool_w", [1, H]), ("pool_b", [1]),
        ("img_w1", [H, ID]), ("img_b1", [H]), ("img_w2", [H, H]), ("img_b2", [H]),
        ("head_w1", [H, 3 * H]), ("head_b1", [H]), ("head_w2", [1, H]), ("head_b2", [1]),
    ]:
        wd[name] = nc.dram_tensor(name, shape, f32, kind="ExternalInput")
    out_d = nc.dram_tensor("out", [1, BS], f32, kind="ExternalOutput")

    # DRAM scratch
    bc_sp = nc.dram_tensor("bc_sp", [2 * DS, BT], bf16)   # B rows 0:16, C rows 16:32
    z_sp = nc.dram_tensor("z_sp", [DI, BT], bf16)         # silu(z) spill
    dt_sp = nc.dram_tensor("dt_sp", [2, 128, BT], bf16)   # dt spill for d=2,3
    dtu_sp = nc.dram_tensor("dtu_sp", [2, 128, BT], bf16)
    st_sp = nc.dram_tensor("st_sp", [2, BT], bf16)        # LN mu/inv bf16 rows
    ex_sp = nc.dram_tensor("ex_sp", [1, BT], bf16)        # attn-pool exp row
    rs_sp = nc.dram_tensor("rs_sp", [1, BS], f32)         # attn-pool 1/sum

    with tile.TileContext(nc) as tc, ExitStack() as ctx:
        wpool = ctx.enter_context(tc.tile_pool(name="wpool", bufs=1))
        ap = ctx.enter_context(tc.tile_pool(name="ap", bufs=2))

        WBUFS = 9

        def wtile(name):
            """Big rotating transient slot [128, <=4120]."""
            return ap.tile([128, BT], bf16, tag="W", bufs=WBUFS, name=name)

        # ---------------- constants ----------------
        ident = wpool.tile([128, 128], bf16, name="ident")
        nc.vector.memset(ident[:], 1.0)
        nc.gpsimd.affine_select(ident[:], ident[:], pattern=[[-1, 128]], base=0,
                                channel_multiplier=1, compare_op=ALU.is_equal, fill=0.0)
        ones_col = wpool.tile([128, 1], bf16, name="ones_col")
        nc.vector.memset(ones_col[:], 1.0)
        eps_col = wpool.tile([128, 1], f32, name="eps_col")
        nc.vector.memset(eps_col[:], 1e-5)

        # ---------------- input DMAs first (own the SP queue head) ----------
        xiT = ap.tile([ID, BS], f32, tag="xiT", name="xiT")
        nc.sync.dma_start(xiT[:], xi_d.ap().rearrange("b f -> f b"))

        # ---------------- weight preprocessing ----------------
        ld_ctx = ExitStack()
        ldp = ld_ctx.enter_context(tc.tile_pool(name="ldp", bufs=3))
        ldps = ld_ctx.enter_context(tc.tile_pool(name="ldps", bufs=2, space="PSUM"))

        def load_cols(src_ap, n, name):
            """1-D DRAM vector [n] -> list of [128,1] f32 col tiles."""
            cols = []
            for blk in range((n + 127) // 128):
                m = min(128, n - blk * 128)
                t = wpool.tile([m, 1], f32, name=f"{name}_c{blk}")
                nc.sync.dma_start(t[:, 0:1],
                                  src_ap[blk * 128: blk * 128 + m].rearrange("(a b) -> a b", b=1))
                cols.append(t)
            return cols

        def load_T(src_ap, R, C, name):
            """DRAM [R, C] f32 -> transposed bf16 SBUF tiles (list over C-blocks of [cm, R]).

            Casting f32->bf16 happens inside the gpsimd software-DGE DMA."""
            nrb = (R + 127) // 128
            ncb = (C + 127) // 128
            outs = []
            for cb in range(ncb):
                cm = min(128, C - cb * 128)
                t = wpool.tile([cm, R], bf16, name=f"{name}_T{cb}")
                outs.append(t)
            for rb in range(nrb):
                rm = min(128, R - rb * 128)
                nat16 = ldp.tile([rm, C], bf16, tag="ld16", name=f"{name}_m{rb}")
                nc.gpsimd.dma_start(nat16[:], src_ap[rb * 128: rb * 128 + rm, :])
                for cb in range(ncb):
                    cm = min(128, C - cb * 128)
                    tp = ldps.tile([cm, rm], bf16, tag="ldT", name=f"{name}_p{rb}_{cb}")
                    nc.tensor.transpose(tp[:], nat16[:, cb * 128: cb * 128 + cm],
                                        ident[0:rm, 0:rm])
                    nc.vector.tensor_copy(outs[cb][:, rb * 128: rb * 128 + rm], tp[:])
            return outs

        ventT = load_T(wd["vent_in_w"].ap(), H, VD, "ventT")          # 1 x [64, 256]
        vent_b = load_cols(wd["vent_in_b"].ap(), H, "vent_b")
        vlnw = load_cols(wd["vent_ln_w"].ap(), H, "vlnw")
        vlnb = load_cols(wd["vent_ln_b"].ap(), H, "vlnb")
        inwT, xpwT, dtwT, outwT = [], [], [], []
        conv_w, conv_b, dt_b, A_t, D_t, lnw, lnb = [], [], [], [], [], [], []
        for l in range(NL):
            inwT.append(load_T(wd["m_in_w"].ap()[l], 2 * DI, H, f"inwT{l}"))
            xpwT.append(load_T(wd["m_xproj_w"].ap()[l], DR + 2 * DS, DI, f"xpwT{l}"))
            dtwT.append(load_T(wd["m_dt_w"].ap()[l], DI, DR, f"dtwT{l}"))
            outwT.append(load_T(wd["m_out_w"].ap()[l], H, DI, f"outwT{l}"))
            cwl, al = [], []
            for d in range(DB):
                sl = slice(d * 128, (d + 1) * 128)
                cw = wpool.tile([128, DC], f32, name=f"cw{l}_{d}")
                nc.sync.dma_start(cw[:], wd["m_conv_w"].ap()[l, sl, :])
                cwl.append(cw)
                alog = ldp.tile([128, DS], f32, tag="alog", name=f"alog{l}_{d}")
                nc.sync.dma_start(alog[:], wd["m_Alog"].ap()[l, sl, :])
                a = wpool.tile([128, DS], f32, name=f"A{l}_{d}")
                nc.scalar.activation(a[:], alog[:], AF.Exp)
                nc.vector.tensor_scalar_mul(a[:], a[:], -1.0)
                al.append(a)
            conv_w.append(cwl)
            conv_b.append(load_cols(wd["m_conv_b"].ap()[l], DI, f"cb{l}"))
            dt_b.append(load_cols(wd["m_dt_b"].ap()[l], DI, f"dtb{l}"))
            Dcols = load_cols(wd["m_D"].ap()[l], DI, f"D{l}")
            dgl = []
            for d in range(DB):
                dg = wpool.tile([128, 128], bf16, name=f"dg{l}_{d}")
                nc.vector.tensor_scalar_mul(dg[:], ident[:], Dcols[d][:, 0:1])
                dgl.append(dg)
            D_t.append(dgl)
            A_t.append(al)
            lnw.append(load_cols(wd["m_ln_w"].ap()[l], H, f"lnw{l}"))
            lnb.append(load_cols(wd["m_ln_b"].ap()[l], H, f"lnb{l}"))
        poolT = load_T(wd["pool_w"].ap(), 1, H, "poolT")              # 2 x [128, 1]
        poolb = wpool.tile([1, 1], f32, name="poolb")
        nc.sync.dma_start(poolb[:], wd["pool_b"].ap().rearrange("(a b) -> a b", b=1))
        imgw1T = load_T(wd["img_w1"].ap(), H, ID, "imgw1T")           # 1 x [32, 256]
        imgb1 = load_cols(wd["img_b1"].ap(), H, "imgb1")
        imgw2T = load_T(wd["img_w2"].ap(), H, H, "imgw2T")            # 2 x [128, 256]
        imgb2 = load_cols(wd["img_b2"].ap(), H, "imgb2")
        h1T = load_T(wd["head_w1"].ap(), H, 3 * H, "h1T")             # 6 x [128, 256]
        hb1 = load_cols(wd["head_b1"].ap(), H, "hb1")
        h2T = load_T(wd["head_w2"].ap(), 1, H, "h2T")                 # 2 x [128, 1]
        hb2 = wpool.tile([1, 1], f32, name="hb2")
        nc.sync.dma_start(hb2[:], wd["head_b2"].ap().rearrange("(a b) -> a b", b=1))
        ld_ctx.close()

        # ---------------- image branch (independent of the mamba stack) -----
        xiT16 = ap.tile([ID, BS], bf16, tag="xiT16", name="xiT16")
        nc.vector.tensor_copy(xiT16[:], xiT[:])
        ii2 = []
        with tc.tile_pool(name="Ips", bufs=2, space="PSUM") as ips:
            ii1 = []
            for hb in range(HB):
                ps = ips.tile([128, BS], f32, tag="hp", name=f"i1p{hb}")
                nc.tensor.matmul(ps[:], imgw1T[0][0:ID, hb * 128:(hb + 1) * 128], xiT16[:],
                                 start=True, stop=True)
                t = ap.tile([128, BS], bf16, tag="ii1t", name=f"ii1_{hb}")
                nc.scalar.activation(t[:], ps[:], AF.Relu, bias=imgb1[hb][:, 0:1])
                ii1.append(t)
            for hb in range(HB):
                ps = ips.tile([128, BS], f32, tag="hp", name=f"i2p{hb}")
                for kb in range(HB):
                    nc.tensor.matmul(ps[:], imgw2T[kb][:, hb * 128:(hb + 1) * 128],
                                     ii1[kb][:], start=(kb == 0), stop=(kb == HB - 1))
                t = ap.tile([128, BS], bf16, tag="ii2t", name=f"ii2_{hb}")
                nc.scalar.activation(t[:], ps[:], AF.Relu, bias=imgb2[hb][:, 0:1])
                ii2.append(t)

        # ---------------- layernorm over H (partition dim) ----------------
        def layernorm(xo, w_cols, b_cols, tag):
            """xo: HB bf16 [128, BT] tiles (pre-norm) -> normalized tiles (tag 'x')."""
            sq = []
            for hb in range(HB):
                sqt = wtile(f"sq_{tag}_{hb}")
                nc.vector.tensor_tensor(sqt[:], xo[hb][:], xo[hb][:], ALU.mult)
                sq.append(sqt)
            mu8 = ap.tile([BS, 512], f32, tag="ln8", bufs=4, name=f"mu8_{tag}")
            msq8 = ap.tile([BS, 512], f32, tag="ln8", bufs=4, name=f"msq8_{tag}")
            with tc.tile_pool(name=f"lnps_{tag}", bufs=2, space="PSUM") as lps:
                for s in range(BS):
                    sl = slice(s * 512, (s + 1) * 512)
                    ps_x = lps.tile([1, 512], f32, tag="lnst1", name=f"sx_{tag}_{s}")
                    for hb in range(HB):
                        nc.tensor.matmul(ps_x[:], ones_col[:], xo[hb][:, sl],
                                         start=(hb == 0), stop=(hb == HB - 1))
                    sxs = ap.tile([1, 512], f32, tag="lnsl", bufs=2, name=f"sxs_{tag}_{s}")
                    nc.scalar.activation(sxs[:], ps_x[:], AF.Copy, scale=1.0 / H)
                    nc.sync.dma_start(mu8[s:s + 1, :], sxs[:])
                    ps_q = lps.tile([1, 512], f32, tag="lnst2", name=f"sq_{tag}_{s}")
                    for hb in range(HB):
                        nc.tensor.matmul(ps_q[:], ones_col[:], sq[hb][:, sl],
                                         start=(hb == 0), stop=(hb == HB - 1))
                    sqs2 = ap.tile([1, 512], f32, tag="lnsl", bufs=2, name=f"sqs_{tag}_{s}")
                    nc.scalar.activation(sqs2[:], ps_q[:], AF.Copy, scale=1.0 / H)
                    nc.sync.dma_start(msq8[s:s + 1, :], sqs2[:])
            var8 = ap.tile([BS, 512], f32, tag="ln8", bufs=4, name=f"var8_{tag}")
            nc.vector.tensor_tensor(var8[:], mu8[:], mu8[:], ALU.mult)
            nc.vector.tensor_tensor(var8[:], msq8[:], var8[:], ALU.subtract)
            sd8 = ap.tile([BS, 512], f32, tag="ln8", bufs=4, name=f"sd8_{tag}")
            nc.scalar.activation(sd8[:], var8[:], AF.Sqrt, bias=eps_col[0:BS, 0:1])
            inv8 = ap.tile([BS, 512], f32, tag="ln8", bufs=4, name=f"inv8_{tag}")
            nc.vector.reciprocal(inv8[:], sd8[:])
            mu16 = ap.tile([BS, 512], bf16, tag="ln16", bufs=2, name=f"mu16_{tag}")
            nc.vector.tensor_copy(mu16[:], mu8[:])
            inv16 = ap.tile([BS, 512], bf16, tag="ln16", bufs=2, name=f"inv16_{tag}")
            nc.vector.tensor_copy(inv16[:], inv8[:])
            nc.sync.dma_start(st_sp.ap()[0, :].rearrange("(b t) -> b t", b=BS), mu16[:])
            nc.sync.dma_start(st_sp.ap()[1, :].rearrange("(b t) -> b t", b=BS), inv16[:])
            mu_rep = wtile(f"murep_{tag}")
            nc.sync.dma_start(mu_rep[:], st_sp.ap()[0, :].partition_broadcast(128))
            inv_rep = wtile(f"invrep_{tag}")
            nc.scalar.dma_start(inv_rep[:], st_sp.ap()[1, :].partition_broadcast(128))
            xcs = []
            for hb in range(HB):
                xc = wtile(f"xc_{tag}_{hb}")
                nc.vector.tensor_tensor(xc[:], xo[hb][:], mu_rep[:], ALU.subtract)
                xcs.append(xc)
            x_out = []
            for hb in range(HB):
                xn = wtile(f"xn_{tag}_{hb}")
                nc.vector.tensor_tensor(xn[:], xcs[hb][:], inv_rep[:], ALU.mult)
                xt = ap.tile([128, BT], bf16, tag="x", bufs=2, name=f"x_{tag}_{hb}")
                nc.scalar.activation(xt[:], xn[:], AF.Identity,
                                     scale=w_cols[hb][:, 0:1], bias=b_cols[hb][:, 0:1])
                x_out.append(xt)
            return x_out

        # ---------------- vent input projection ----------------
        xvT = wtile("xvT")  # [64, BT] bf16 on first 64 partitions
        xo0 = []
        for half in range(2):
            xvTf = ap.tile([VD, BT // 2], f32, tag="W", bufs=WBUFS, name=f"xvTf{half}")
            nc.sync.dma_start_transpose(
                xvTf[:], xv_d.ap().rearrange("b l v -> (b l) v")[half * 2048:(half + 1) * 2048, :])
            nc.vector.tensor_copy(xvT[0:VD, half * 2048:(half + 1) * 2048], xvTf[:])
        with tc.tile_pool(name="ventps", bufs=3, space="PSUM") as vps:
            for hb in range(HB):
                xo_t = wtile(f"vxo{hb}")
                for s in range(BS):
                    sl = slice(s * 512, (s + 1) * 512)
                    ps = vps.tile([128, 512], f32, tag="pj", name=f"vps{hb}_{s}")
                    nc.tensor.matmul(ps[:], ventT[0][:, hb * 128:(hb + 1) * 128],
                                     xvT[0:VD, sl], start=True, stop=True)
                    nc.scalar.activation(xo_t[:, sl], ps[:], AF.Identity,
                                         bias=vent_b[hb][:, 0:1])
                xo0.append(xo_t)
        x = layernorm(xo0, vlnw, vlnb, "vent")

        # ---------------- mamba layers ----------------
        for l in range(NL):
            u_t = []
            # ---- phase A (u half) + depthwise causal conv + silu ----
            with tc.tile_pool(name=f"Aps{l}", bufs=3, space="PSUM") as aps:
                for d in range(DB):
                    ur = ap.tile([128, PBT], bf16, tag="W", bufs=WBUFS, name=f"uraw{l}_{d}")
                    for b in range(BS):
                        nc.gpsimd.memset(ur[:, b * LP: b * LP + DC - 1], 0.0)
                    for s in range(BS):
                        sl = slice(s * 512, (s + 1) * 512)
                        ps = aps.tile([128, 512], f32, tag="pj", name=f"aps{l}_{d}_{s}")
                        for kb in range(HB):
                            nc.tensor.matmul(ps[:], inwT[l][kb][:, d * 128:(d + 1) * 128],
                                             x[kb][:, sl], start=(kb == 0), stop=(kb == HB - 1))
                        nc.scalar.activation(ur[:, s * LP + DC - 1:(s + 1) * LP], ps[:],
                                             AF.Copy)
                    # conv: full-width shifted 2D slices (stay inside each
                    # 515-seg). ACT does the per-channel scaled shifts (it has
                    # slack); DVE does packed-2x pairwise adds.
                    CW = PBT - DC + 1
                    sh = []
                    for k in range(DC):
                        st = ap.tile([128, PBT], bf16, tag="W", bufs=WBUFS,
                                     name=f"csh{l}_{d}_{k}")
                        nc.vector.tensor_scalar_mul(st[:, 0:CW], ur[:, k:CW + k],
                                                    conv_w[l][d][:, k:k + 1])
                        sh.append(st)
                    a01 = ap.tile([128, PBT], bf16, tag="W", bufs=WBUFS,
                                  name=f"ca01{l}_{d}")
                    nc.vector.tensor_tensor(a01[:, 0:CW], sh[0][:, 0:CW],
                                            sh[1][:, 0:CW], ALU.add)
                    a23 = ap.tile([128, PBT], bf16, tag="W", bufs=WBUFS,
                                  name=f"ca23{l}_{d}")
                    nc.vector.tensor_tensor(a23[:, 0:CW], sh[2][:, 0:CW],
                                            sh[3][:, 0:CW], ALU.add)
                    acc = ap.tile([128, PBT], bf16, tag="W", bufs=WBUFS,
                                  name=f"cacc{l}_{d}")
                    nc.vector.tensor_tensor(acc[:, 0:CW], a01[:, 0:CW],
                                            a23[:, 0:CW], ALU.add)
                    ut = ap.tile([128, BT], bf16, tag="u", bufs=4, name=f"u{l}_{d}")
                    nc.scalar.activation(
                        ut[:].rearrange("p (b t) -> p b t", b=BS),
                        acc[:].rearrange("p (b t) -> p b t", t=LP)[:, :, 0:512],
                        AF.Silu, bias=conv_b[l][d][:, 0:1])
                    u_t.append(ut)
                    # z quarter for this d: fills the PE gap left by conv
                    mb = 4 + d
                    zt = wtile(f"z{l}_{mb}")
                    for s in range(BS):
                        sl = slice(s * 512, (s + 1) * 512)
                        ps = aps.tile([128, 512], f32, tag="pj", name=f"zps{l}_{mb}_{s}")
                        for kb in range(HB):
                            nc.tensor.matmul(ps[:], inwT[l][kb][:, mb * 128:(mb + 1) * 128],
                                             x[kb][:, sl], start=(kb == 0), stop=(kb == HB - 1))
                        nc.vector.tensor_copy(zt[:, sl], ps[:])
                    nc.scalar.dma_start(z_sp.ap()[(mb - 4) * 128:(mb - 3) * 128, :], zt[:])
            # ---- phase C: xproj -> (B | C) rows first (unblocks the E-phase
            # broadcast pipeline), then the dt_in rows ----
            xdb = ap.tile([64, BT], bf16, tag="xd", bufs=1, name=f"xdb{l}")
            xdt = xdb[0:16, :]
            xbc = xdb[32:64, :]
            with tc.tile_pool(name=f"Cps{l}", bufs=3, space="PSUM") as cps:
                for s in range(BS):
                    sl = slice(s * 512, (s + 1) * 512)
                    ps = cps.tile([16, 512], f32, tag="pdt", name=f"cpd{l}_{s}")
                    for kb in range(DB):
                        nc.tensor.matmul(ps[:], xpwT[l][kb][:, 0:16], u_t[kb][:, sl],
                                         start=(kb == 0), stop=(kb == DB - 1))
                    nc.vector.tensor_copy(xdt[:, sl], ps[:])
                for s in range(BS):
                    sl = slice(s * 512, (s + 1) * 512)
                    ps = cps.tile([32, 512], f32, tag="pbc", name=f"cps{l}_{s}")
                    for kb in range(DB):
                        nc.tensor.matmul(ps[:], xpwT[l][kb][:, 16:48], u_t[kb][:, sl],
                                         start=(kb == 0), stop=(kb == DB - 1))
                    nc.vector.tensor_copy(xbc[:, sl], ps[:])
                nc.sync.dma_start(bc_sp.ap()[:, :], xbc[:, :])

            # ---- phase D: dt_proj -> softplus; dtu; poison; spill d>=2 ----
            dt_res, dtu_res = {}, {}
            with tc.tile_pool(name=f"Dps{l}", bufs=3, space="PSUM") as dps:
                for d in range(DB):
                    et = wtile(f"et{l}_{d}")
                    for s in range(BS):
                        sl = slice(s * 512, (s + 1) * 512)
                        ps = dps.tile([128, 512], f32, tag="pj", name=f"dps{l}_{d}_{s}")
                        nc.tensor.matmul(ps[:], dtwT[l][0][0:DR, d * 128:(d + 1) * 128],
                                         xdt[0:DR, sl], start=True, stop=True)
                        # softplus(x+b) = ln(1 + exp(x+b))
                        nc.scalar.activation(et[:, sl], ps[:], AF.Exp,
                                             bias=dt_b[l][d][:, 0:1])
                    if d < 2:
                        dt_t = ap.tile([128, BT], bf16, tag="dt", bufs=2, name=f"dt{l}_{d}")
                    else:
                        dt_t = wtile(f"dtw{l}_{d}")
                    nc.scalar.activation(dt_t[:], et[:], AF.Ln, bias=1.0)
                    if d < 2:
                        dtu = ap.tile([128, BT], bf16, tag="dtu", bufs=2, name=f"dtu{l}_{d}")
                    else:
                        dtu = wtile(f"dtuw{l}_{d}")
                    nc.vector.tensor_tensor(dtu[:], dt_t[:], u_t[d][:], ALU.mult)
                    for b in range(BS):
                        nc.gpsimd.memset(dt_t[:, b * L: b * L + 1], POISON)
                    if d >= 2:
                        nc.scalar.dma_start(dt_sp.ap()[d - 2], dt_t[:])
                        nc.scalar.dma_start(dtu_sp.ap()[d - 2], dtu[:])
                    else:
                        dt_res[d] = dt_t
                        dtu_res[d] = dtu

            # ---- phase E: selective scan ----
            with tc.tile_pool(name=f"Eps{l}", bufs=1, space="PSUM") as eps_pool:
                for d in range(DB):
                    if d < 2:
                        dtL, dtuL = dt_res[d], dtu_res[d]
                    else:
                        dtL = ap.tile([128, BT], bf16, tag="dt", bufs=2, name=f"dtL{l}_{d}")
                        nc.scalar.dma_start(dtL[:], dt_sp.ap()[d - 2])
                        dtuL = ap.tile([128, BT], bf16, tag="dtu", bufs=2, name=f"dtuL{l}_{d}")
                        nc.scalar.dma_start(dtuL[:], dtu_sp.ap()[d - 2])
                    y_ps = eps_pool.tile([128, BT], f32, tag="y", name=f"yps{l}_{d}")
                    # skip term first: y = diag(D) @ u  (so the accumulation
                    # finishes right after the last state's idents)
                    for si in range(BS):
                        sl = slice(si * 512, (si + 1) * 512)
                        nc.tensor.matmul(y_ps[:, sl], D_t[l][d][:], u_t[d][:, sl],
                                         start=True, stop=False)
                    zs = None
                    for n in range(DS):
                        repB = wtile(f"rb{l}_{d}_{n}")
                        nc.sync.dma_start(repB[:], bc_sp.ap()[n, :].partition_broadcast(128))
                        repC = wtile(f"rc{l}_{d}_{n}")
                        nc.gpsimd.dma_start(repC[:],
                                            bc_sp.ap()[DS + n, :].partition_broadcast(128))
                        dA = wtile(f"dA{l}_{d}_{n}")
                        nc.scalar.activation(dA[:], dtL[:], AF.Exp,
                                             scale=A_t[l][d][:, n:n + 1])
                        dBu = wtile(f"dBu{l}_{d}_{n}")
                        nc.vector.tensor_tensor(dBu[:], dtuL[:], repB[:], ALU.mult)
                        h = wtile(f"h{l}_{d}_{n}")
                        nc.vector.tensor_tensor_scan(h[:], dA[:], dBu[:], 0.0,
                                                     ALU.mult, ALU.add)
                        hc = wtile(f"hc{l}_{d}_{n}")
                        nc.vector.tensor_tensor(hc[:], h[:], repC[:], ALU.mult)
                        for si in range(BS):
                            sl = slice(si * 512, (si + 1) * 512)
                            nc.tensor.matmul(y_ps[:, sl], ident[:], hc[:, sl],
                                             start=False, stop=(n == DS - 1))
                        if n == DS - 2:
                            # prefetch + silu the gate input during the last unit
                            zsr = wtile(f"zsr{l}_{d}")
                            nc.sync.dma_start(zsr[:],
                                              z_sp.ap()[d * 128:(d + 1) * 128, :])
                            zs = wtile(f"zs{l}_{d}")
                            nc.scalar.activation(zs[:], zsr[:], AF.Silu)
                    # gate per 512-chunk, pipelined behind the last ident pass
                    for si in range(BS):
                        sl = slice(si * 512, (si + 1) * 512)
                        nc.vector.tensor_tensor(u_t[d][:, sl], zs[:, sl], y_ps[:, sl],
                                                ALU.mult)

            # ---- phase F: out_proj + layernorm ----
            xo = []
            with tc.tile_pool(name=f"Fps{l}", bufs=3, space="PSUM") as fps:
                for hb in range(HB):
                    xo_t = wtile(f"xo{l}_{hb}")
                    for s in range(BS):
                        sl = slice(s * 512, (s + 1) * 512)
                        ps = fps.tile([128, 512], f32, tag="pj", name=f"fps{l}_{hb}_{s}")
                        for kb in range(DB):
                            nc.tensor.matmul(ps[:], outwT[l][kb][:, hb * 128:(hb + 1) * 128],
                                             u_t[kb][:, sl], start=(kb == 0),
                                             stop=(kb == DB - 1))
                        nc.scalar.activation(xo_t[:, sl], ps[:], AF.Copy)
                    xo.append(xo_t)
            x = layernorm(xo, lnw[l], lnb[l], f"l{l}")

        # ---------------- attention pool over time (softmax, no max-sub:
        # logits are O(0.3) so exp is perfectly stable) ----------------
        ex16 = wtile("ex16")  # [1, BT] used on partition 0
        with tc.tile_pool(name="Pps", bufs=3, space="PSUM") as pps:
            for s in range(BS):
                sl = slice(s * 512, (s + 1) * 512)
                ps = pps.tile([1, 512], f32, tag="lgst", name=f"pps{s}")
                for hb in range(HB):
                    nc.tensor.matmul(ps[:], poolT[hb][:, 0:1], x[hb][:, sl],
                                     start=(hb == 0), stop=(hb == HB - 1))
                nc.scalar.activation(ex16[0:1, sl], ps[:], AF.Exp, bias=poolb[0:1, 0:1])
        sm8 = ap.tile([1, BS], f32, tag="smc", name="sm8")
        nc.vector.tensor_reduce(sm8[:], ex16[0:1, :].rearrange("p (b t) -> p b t", b=BS),
                                axis=AX.X, op=ALU.add)
        rs = ap.tile([1, BS], f32, tag="smc", name="rs")
        nc.vector.reciprocal(rs[:], sm8[:])
        nc.sync.dma_start(rs_sp.ap(), rs[:])
        nc.sync.dma_start(ex_sp.ap(), ex16[0:1, :])
        ex_rep = wtile("ex_rep")
        nc.sync.dma_start(ex_rep[:], ex_sp.ap()[0, :].partition_broadcast(128))
        rs_rep = ap.tile([128, BS], f32, tag="rsr", name="rs_rep")
        nc.sync.dma_start(rs_rep[:], rs_sp.ap()[0, :].partition_broadcast(128))
        v_t = []
        for hb in range(HB):
            xa = wtile(f"xa{hb}")
            nc.vector.tensor_tensor(xa[:], x[hb][:], ex_rep[:], ALU.mult)
            vv = ap.tile([128, BS], f32, tag="vsm", bufs=2, name=f"vv{hb}")
            nc.vector.tensor_reduce(vv[:], xa[:].rearrange("p (b t) -> p b t", b=BS),
                                    axis=AX.X, op=ALU.add)
            v16 = ap.tile([128, BS], bf16, tag="vshb", name=f"v16_{hb}")
            nc.vector.tensor_tensor(v16[:], vv[:], rs_rep[:], ALU.mult)
            v_t.append(v16)

        # ---------------- fusion head ----------------
        with tc.tile_pool(name="Hps", bufs=3, space="PSUM") as hps:
            vi = []
            for hb in range(HB):
                t = ap.tile([128, BS], bf16, tag="vit", name=f"vi{hb}")
                nc.vector.tensor_tensor(t[:], v_t[hb][:], ii2[hb][:], ALU.mult)
                vi.append(t)
            f_rhs = [v_t[0], v_t[1], ii2[0], ii2[1], vi[0], vi[1]]
            hh = []
            for mb in range(HB):
                ps = hps.tile([128, BS], f32, tag="hp", name=f"h1p{mb}")
                for kb in range(6):
                    nc.tensor.matmul(ps[:], h1T[kb][:, mb * 128:(mb + 1) * 128],
                                     f_rhs[kb][:], start=(kb == 0), stop=(kb == 5))
                t = ap.tile([128, BS], bf16, tag="hht", name=f"hh{mb}")
                nc.scalar.activation(t[:], ps[:], AF.Relu, bias=hb1[mb][:, 0:1])
                hh.append(t)
            ps = hps.tile([1, BS], f32, tag="hpo", name="outp")
            for kb in range(HB):
                nc.tensor.matmul(ps[:], h2T[kb][:, 0:1], hh[kb][:],
                                 start=(kb == 0), stop=(kb == HB - 1))
            o_sb = ap.tile([1, BS], f32, tag="osb", name="o_sb")
            nc.scalar.activation(o_sb[:], ps[:], AF.Identity, bias=hb2[0:1, 0:1])
        nc.sync.dma_start(out_d.ap(), o_sb[:])

    nc.compile()
    return nc


_NC = None


def _get_nc():
    global _NC
    if _NC is None:
        _NC = _build()
    return _NC


def run(inputs, trace=False):
    nc = _get_nc()
    inputs = {k: np.asarray(v, dtype=np.float32) for k, v in inputs.items()}
    in_maps = []
    for c in range(NCORES):
        m = {name: inputs[name] for name in WEIGHT_NAMES}
        m["xv"] = np.ascontiguousarray(inputs["xv"][c * BS:(c + 1) * BS])
        m["xi"] = np.ascontiguousarray(inputs["xi"][c * BS:(c + 1) * BS])
        in_maps.append(m)
    res = run_bass_kernel_spmd(nc, in_maps, core_ids=list(range(NCORES)), trace=trace)
    out = np.concatenate([np.asarray(res.results[c]["out"]).reshape(BS)
                          for c in range(NCORES)])
    return out.reshape(B, 1).astype(np.float32), res.exec_time_ns


def kernel(**inputs):
    return run(inputs, trace=False)[0]


# revision 47
# speedup vs baseline: 1.0587x; 1.0587x over previous
"""Trainium2 Bass kernel for nn_CrossFusionMamba (2-layer Mamba stack + fusion head).

Self-contained: hardcodes all shapes/sharding. Data-parallel over batch across
8 NeuronCores (8 batch elements per core).

Layout: channels on SBUF partitions, flattened (batch, time) on the free dim
(bt = b*512 + t -> 4096 columns per core). Full-BT tiles everywhere.

Engine assignment for the selective scan (the bottleneck):
  ACT   : dA = exp(A[d,n] * dt)            (per-partition scale)
  DVE   : dBu = (dt*u) . B_n ; h = tensor_tensor_scan(dA, dBu) ; hc = h . C_n
  PE    : y = diag(D) @ u + sum_n I @ hc_n (PSUM accumulation, skip term first)
GpSimd does NO compute: running Pool tensor_tensor concurrently with DVE
tensor_tensor degrades BOTH ~2-4x (SBUF port contention) — measured, twice.
DVE tensor_tensor in packed bf16 SBUF hits the 2x mode (~0.56 ns/col); the
scan runs at ~2.1 ns/col + ~2us fixed, which is the hard floor here.
Batch independence inside one scan op is enforced by poisoning dt at each
batch's first column (dt=1e9 -> dA=exp(-big)=0 -> exact state reset).

B/C rows are spilled to DRAM once per layer and partition-broadcast to
[128, BT] tiles per (d, n) (SP queue for B, gpsimd SWDGE for C).
Weights are loaded via gpsimd casting DMAs (f32->bf16) and PE transposes.
Big transients share one 9-deep rotating slot tag ("W"); slot-reuse WAR
edges are safe because every W tile's readers are emitted within 9
subsequent W allocations.
"""
import sys

if "/opt/trn_rl_repo" not in sys.path:
    sys.path.insert(0, "/opt/trn_rl_repo")

from contextlib import ExitStack

import numpy as np

import concourse.bacc as bacc
import concourse.tile as tile
import concourse.mybir as mybir
from concourse.bass_utils import run_bass_kernel_spmd

f32 = mybir.dt.float32
bf16 = mybir.dt.bfloat16
AF = mybir.ActivationFunctionType
ALU = mybir.AluOpType
AX = mybir.AxisListType

# model dims
B, L, VD, ID = 64, 512, 64, 32
H, DI, DS, DC, DR, NL = 256, 512, 16, 4, 16, 2
NCORES = 8
BS = B // NCORES          # batches per core
BT = BS * L               # free columns per core (4096)
LP = L + DC - 1           # padded per-batch length for conv (515)
PBT = BS * LP             # 4120
HB = H // 128             # 2
DB = DI // 128            # 4
POISON = 1.0e9


WEIGHT_NAMES = [
    "vent_in_w", "vent_in_b", "vent_ln_w", "vent_ln_b",
    "m_in_w", "m_conv_w", "m_conv_b", "m_xproj_w", "m_dt_w", "m_dt_b",
    "m_Alog", "m_D", "m_out_w", "m_ln_w", "m_ln_b",
    "pool_w", "pool_b", "img_w1", "img_b1", "img_w2", "img_b2",
    "head_w1", "head_b1", "head_w2", "head_b2",
]


def _build():
    nc = bacc.Bacc("TRN2", target_bir_lowering=False, debug=False)

    # ---- DRAM I/O ----
    xv_d = nc.dram_tensor("xv", [BS, L, VD], f32, kind="ExternalInput")
    xi_d = nc.dram_tensor("xi", [BS, ID], f32, kind="ExternalInput")
    wd = {}
    for name, shape in [
        ("vent_in_w", [H, VD]), ("vent_in_b", [H]), ("vent_ln_w", [H]), ("vent_ln_b", [H]),
        ("m_in_w", [NL, 2 * DI, H]), ("m_conv_w", [NL, DI, DC]), ("m_conv_b", [NL, DI]),
        ("m_xproj_w", [NL, DR + 2 * DS, DI]), ("m_dt_w", [NL, DI, DR]), ("m_dt_b", [NL, DI]),
        ("m_Alog", [NL, DI, DS]), ("m_D", [NL, DI]), ("m_out_w", [NL, H, DI]),
        ("m_ln_w", [NL, H]), ("m_ln_b", [NL, H]),
        ("pool_w", [1, H]), ("pool_b", [1]),
        ("img_w1", [H, ID]), ("img_b1", [H]), ("img_w2", [H, H]), ("img_b2", [H]),
        ("head_w1", [H, 3 * H]), ("head_b1", [H]), ("head_w2", [1, H]), ("head_b2", [1]),
    ]:
        wd[name] = nc.dram_tensor(name, shape, f32, kind="ExternalInput")
    out_d = nc.dram_tensor("out", [1, BS], f32, kind="ExternalOutput")

    # DRAM scratch
    bc_sp = nc.dram_tensor("bc_sp", [2 * DS, BT], bf16)   # B rows 0:16, C rows 16:32
    z_sp = nc.dram_tensor("z_sp", [DI, BT], bf16)         # silu(z) spill
    dt_sp = nc.dram_tensor("dt_sp", [2, 128, BT], bf16)   # dt spill for d=2,3
    dtu_sp = nc.dram_tensor("dtu_sp", [2, 128, BT], bf16)
    st_sp = nc.dram_tensor("st_sp", [2, BT], bf16)        # LN mu/inv bf16 rows
    ex_sp = nc.dram_tensor("ex_sp", [1, BT], bf16)        # attn-pool exp row
    rs_sp = nc.dram_tensor("rs_sp", [1, BS], f32)         # attn-pool 1/sum

    with tile.TileContext(nc) as tc, ExitStack() as ctx:
        wpool = ctx.enter_context(tc.tile_pool(name="wpool", bufs=1))
        ap = ctx.enter_context(tc.tile_pool(name="ap", bufs=2))

        WBUFS = 9

        def wtile(name):
            """Big rotating transient slot [128, <=4120]."""
            return ap.tile([128, BT], bf16, tag="W", bufs=WBUFS, name=name)

        # ---------------- constants ----------------
        ident = wpool.tile([128, 128], bf16, name="ident")
        nc.vector.memset(ident[:], 1.0)
        nc.gpsimd.affine_select(ident[:], ident[:], pattern=[[-1, 128]], base=0,
                                channel_multiplier=1, compare_op=ALU.is_equal, fill=0.0)
        ones_col = wpool.tile([128, 1], bf16, name="ones_col")
        nc.vector.memset(ones_col[:], 1.0)
        eps_col = wpool.tile([128, 1], f32, name="eps_col")
        nc.vector.memset(eps_col[:], 1e-5)

        # ---------------- input DMAs first (own the SP queue head) ----------
        xiT = ap.tile([ID, BS], f32, tag="xiT", name="xiT")
        nc.sync.dma_start(xiT[:], xi_d.ap().rearrange("b f -> f b"))

        xv16 = ap.tile([128, BS * L * VD // 128], bf16, tag="W", bufs=WBUFS, name="xv16")
        xv_3d = xv_d.ap().rearrange("b l v -> (b l) v").rearrange("(c p) v -> p c v", p=128)
        nc.gpsimd.dma_start(
            xv16[:].rearrange("p (c v) -> p c v", v=VD), xv_3d)

        # ---------------- weight preprocessing ----------------
        ld_ctx = ExitStack()
        ldp = ld_ctx.enter_context(tc.tile_pool(name="ldp", bufs=3))
        ldps = ld_ctx.enter_context(tc.tile_pool(name="ldps", bufs=2, space="PSUM"))

        def load_cols(src_ap, n, name):
            """1-D DRAM vector [n] -> list of [128,1] f32 col tiles."""
            cols = []
            for blk in range((n + 127) // 128):
                m = min(128, n - blk * 128)
                t = wpool.tile([m, 1], f32, name=f"{name}_c{blk}")
                nc.sync.dma_start(t[:, 0:1],
                                  src_ap[blk * 128: blk * 128 + m].rearrange("(a b) -> a b", b=1))
                cols.append(t)
            return cols

        def load_T(src_ap, R, C, name):
            """DRAM [R, C] f32 -> transposed bf16 SBUF tiles (list over C-blocks of [cm, R]).

            Casting f32->bf16 happens inside the gpsimd software-DGE DMA."""
            nrb = (R + 127) // 128
            ncb = (C + 127) // 128
            outs = []
            for cb in range(ncb):
                cm = min(128, C - cb * 128)
                t = wpool.tile([cm, R], bf16, name=f"{name}_T{cb}")
                outs.append(t)
            for rb in range(nrb):
                rm = min(128, R - rb * 128)
                nat16 = ldp.tile([rm, C], bf16, tag="ld16", name=f"{name}_m{rb}")
                nc.gpsimd.dma_start(nat16[:], src_ap[rb * 128: rb * 128 + rm, :])
                for cb in range(ncb):
                    cm = min(128, C - cb * 128)
                    tp = ldps.tile([cm, rm], bf16, tag="ldT", name=f"{name}_p{rb}_{cb}")
                    nc.tensor.transpose(tp[:], nat16[:, cb * 128: cb * 128 + cm],
                                        ident[0:rm, 0:rm])
                    nc.vector.tensor_copy(outs[cb][:, rb * 128: rb * 128 + rm], tp[:])
            return outs

        ventT = load_T(wd["vent_in_w"].ap(), H, VD, "ventT")          # 1 x [64, 256]
        vent_b = load_cols(wd["vent_in_b"].ap(), H, "vent_b")
        vlnw = load_cols(wd["vent_ln_w"].ap(), H, "vlnw")
        vlnb = load_cols(wd["vent_ln_b"].ap(), H, "vlnb")
        inwT, xpwT, dtwT, outwT = [], [], [], []
        conv_w, conv_b, dt_b, A_t, D_t, lnw, lnb = [], [], [], [], [], [], []
        for l in range(NL):
            inwT.append(load_T(wd["m_in_w"].ap()[l], 2 * DI, H, f"inwT{l}"))
            xpwT.append(load_T(wd["m_xproj_w"].ap()[l], DR + 2 * DS, DI, f"xpwT{l}"))
            dtwT.append(load_T(wd["m_dt_w"].ap()[l], DI, DR, f"dtwT{l}"))
            outwT.append(load_T(wd["m_out_w"].ap()[l], H, DI, f"outwT{l}"))
            cwl, al = [], []
            for d in range(DB):
                sl = slice(d * 128, (d + 1) * 128)
                cw = wpool.tile([128, DC], f32, name=f"cw{l}_{d}")
                nc.sync.dma_start(cw[:], wd["m_conv_w"].ap()[l, sl, :])
                cwl.append(cw)
                alog = ldp.tile([128, DS], f32, tag="alog", name=f"alog{l}_{d}")
                nc.sync.dma_start(alog[:], wd["m_Alog"].ap()[l, sl, :])
                a = wpool.tile([128, DS], f32, name=f"A{l}_{d}")
                nc.scalar.activation(a[:], alog[:], AF.Exp)
                nc.vector.tensor_scalar_mul(a[:], a[:], -1.0)
                al.append(a)
            conv_w.append(cwl)
            conv_b.append(load_cols(wd["m_conv_b"].ap()[l], DI, f"cb{l}"))
            dt_b.append(load_cols(wd["m_dt_b"].ap()[l], DI, f"dtb{l}"))
            Dcols = load_cols(wd["m_D"].ap()[l], DI, f"D{l}")
            dgl = []
            for d in range(DB):
                dg = wpool.tile([128, 128], bf16, name=f"dg{l}_{d}")
                nc.vector.tensor_scalar_mul(dg[:], ident[:], Dcols[d][:, 0:1])
                dgl.append(dg)
            D_t.append(dgl)
            A_t.append(al)
            lnw.append(load_cols(wd["m_ln_w"].ap()[l], H, f"lnw{l}"))
            lnb.append(load_cols(wd["m_ln_b"].ap()[l], H, f"lnb{l}"))
        poolT = load_T(wd["pool_w"].ap(), 1, H, "poolT")              # 2 x [128, 1]
        poolb = wpool.tile([1, 1], f32, name="poolb")
        nc.sync.dma_start(poolb[:], wd["pool_b"].ap().rearrange("(a b) -> a b", b=1))
        imgw1T = load_T(wd["img_w1"].ap(), H, ID, "imgw1T")           # 1 x [32, 256]
        imgb1 = load_cols(wd["img_b1"].ap(), H, "imgb1")
        imgw2T = load_T(wd["img_w2"].ap(), H, H, "imgw2T")            # 2 x [128, 256]
        imgb2 = load_cols(wd["img_b2"].ap(), H, "imgb2")
        h1T = load_T(wd["head_w1"].ap(), H, 3 * H, "h1T")             # 6 x [128, 256]
        hb1 = load_cols(wd["head_b1"].ap(), H, "hb1")
        h2T = load_T(wd["head_w2"].ap(), 1, H, "h2T")                 # 2 x [128, 1]
        hb2 = wpool.tile([1, 1], f32, name="hb2")
        nc.sync.dma_start(hb2[:], wd["head_b2"].ap().rearrange("(a b) -> a b", b=1))
        ld_ctx.close()

        # ---------------- image branch (independent of the mamba stack) -----
        xiT16 = ap.tile([ID, BS], bf16, tag="xiT16", name="xiT16")
        nc.vector.tensor_copy(xiT16[:], xiT[:])
        ii2 = []
        with tc.tile_pool(name="Ips", bufs=2, space="PSUM") as ips:
            ii1 = []
            for hb in range(HB):
                ps = ips.tile([128, BS], f32, tag="hp", name=f"i1p{hb}")
                nc.tensor.matmul(ps[:], imgw1T[0][0:ID, hb * 128:(hb + 1) * 128], xiT16[:],
                                 start=True, stop=True)
                t = ap.tile([128, BS], bf16, tag="ii1t", name=f"ii1_{hb}")
                nc.scalar.activation(t[:], ps[:], AF.Relu, bias=imgb1[hb][:, 0:1])
                ii1.append(t)
            for hb in range(HB):
                ps = ips.tile([128, BS], f32, tag="hp", name=f"i2p{hb}")
                for kb in range(HB):
                    nc.tensor.matmul(ps[:], imgw2T[kb][:, hb * 128:(hb + 1) * 128],
                                     ii1[kb][:], start=(kb == 0), stop=(kb == HB - 1))
                t = ap.tile([128, BS], bf16, tag="ii2t", name=f"ii2_{hb}")
                nc.scalar.activation(t[:], ps[:], AF.Relu, bias=imgb2[hb][:, 0:1])
                ii2.append(t)

        # ---------------- layernorm over H (partition dim) ----------------
        def layernorm(xo, w_cols, b_cols, tag):
            """xo: HB bf16 [128, BT] tiles (pre-norm) -> normalized tiles (tag 'x')."""
            sq = []
            for hb in range(HB):
                sqt = wtile(f"sq_{tag}_{hb}")
                nc.vector.tensor_tensor(sqt[:], xo[hb][:], xo[hb][:], ALU.mult)
                sq.append(sqt)
            mu8 = ap.tile([BS, 512], f32, tag="ln8", bufs=4, name=f"mu8_{tag}")
            msq8 = ap.tile([BS, 512], f32, tag="ln8", bufs=4, name=f"msq8_{tag}")
            with tc.tile_pool(name=f"lnps_{tag}", bufs=2, space="PSUM") as lps:
                for s in range(BS):
                    sl = slice(s * 512, (s + 1) * 512)
                    ps_x = lps.tile([1, 512], f32, tag="lnst1", name=f"sx_{tag}_{s}")
                    for hb in range(HB):
                        nc.tensor.matmul(ps_x[:], ones_col[:], xo[hb][:, sl],
                                         start=(hb == 0), stop=(hb == HB - 1))
                    sxs = ap.tile([1, 512], f32, tag="lnsl", bufs=2, name=f"sxs_{tag}_{s}")
                    nc.scalar.activation(sxs[:], ps_x[:], AF.Copy, scale=1.0 / H)
                    nc.sync.dma_start(mu8[s:s + 1, :], sxs[:])
                    ps_q = lps.tile([1, 512], f32, tag="lnst2", name=f"sq_{tag}_{s}")
                    for hb in range(HB):
                        nc.tensor.matmul(ps_q[:], ones_col[:], sq[hb][:, sl],
                                         start=(hb == 0), stop=(hb == HB - 1))
                    sqs2 = ap.tile([1, 512], f32, tag="lnsl", bufs=2, name=f"sqs_{tag}_{s}")
                    nc.scalar.activation(sqs2[:], ps_q[:], AF.Copy, scale=1.0 / H)
                    nc.sync.dma_start(msq8[s:s + 1, :], sqs2[:])
            var8 = ap.tile([BS, 512], f32, tag="ln8", bufs=4, name=f"var8_{tag}")
            nc.vector.tensor_tensor(var8[:], mu8[:], mu8[:], ALU.mult)
            nc.vector.tensor_tensor(var8[:], msq8[:], var8[:], ALU.subtract)
            sd8 = ap.tile([BS, 512], f32, tag="ln8", bufs=4, name=f"sd8_{tag}")
            nc.scalar.activation(sd8[:], var8[:], AF.Sqrt, bias=eps_col[0:BS, 0:1])
            inv8 = ap.tile([BS, 512], f32, tag="ln8", bufs=4, name=f"inv8_{tag}")
            nc.vector.reciprocal(inv8[:], sd8[:])
            mu16 = ap.tile([BS, 512], bf16, tag="ln16", bufs=2, name=f"mu16_{tag}")
            nc.vector.tensor_copy(mu16[:], mu8[:])
            inv16 = ap.tile([BS, 512], bf16, tag="ln16", bufs=2, name=f"inv16_{tag}")
            nc.vector.tensor_copy(inv16[:], inv8[:])
            nc.sync.dma_start(st_sp.ap()[0, :].rearrange("(b t) -> b t", b=BS), mu16[:])
            nc.sync.dma_start(st_sp.ap()[1, :].rearrange("(b t) -> b t", b=BS), inv16[:])
            mu_rep = wtile(f"murep_{tag}")
            nc.sync.dma_start(mu_rep[:], st_sp.ap()[0, :].partition_broadcast(128))
            inv_rep = wtile(f"invrep_{tag}")
            nc.scalar.dma_start(inv_rep[:], st_sp.ap()[1, :].partition_broadcast(128))
            xcs = []
            for hb in range(HB):
                xc = wtile(f"xc_{tag}_{hb}")
                nc.vector.tensor_tensor(xc[:], xo[hb][:], mu_rep[:], ALU.subtract)
                xcs.append(xc)
            x_out = []
            for hb in range(HB):
                xn = wtile(f"xn_{tag}_{hb}")
                nc.vector.tensor_tensor(xn[:], xcs[hb][:], inv_rep[:], ALU.mult)
                xt = ap.tile([128, BT], bf16, tag="x", bufs=2, name=f"x_{tag}_{hb}")
                nc.scalar.activation(xt[:], xn[:], AF.Identity,
                                     scale=w_cols[hb][:, 0:1], bias=b_cols[hb][:, 0:1])
                x_out.append(xt)
            return x_out

        # ---------------- vent input projection ----------------
        xvT = wtile("xvT")  # [64, BT] on first 64 partitions
        xo0 = []
        with tc.tile_pool(name="xvps", bufs=3, space="PSUM") as xps, \
             tc.tile_pool(name="ventps", bufs=3, space="PSUM") as vps:
            for blk in range(BT // 128):
                tp = xps.tile([VD, 128], bf16, tag="xvT", name=f"xvp{blk}")
                nc.tensor.transpose(tp[:], xv16[:, blk * VD:(blk + 1) * VD], ident[:])
                nc.vector.tensor_copy(xvT[0:VD, blk * 128:(blk + 1) * 128], tp[:])
            for hb in range(HB):
                xo_t = wtile(f"vxo{hb}")
                for s in range(BS):
                    sl = slice(s * 512, (s + 1) * 512)
                    ps = vps.tile([128, 512], f32, tag="pj", name=f"vps{hb}_{s}")
                    nc.tensor.matmul(ps[:], ventT[0][:, hb * 128:(hb + 1) * 128],
                                     xvT[0:VD, sl], start=True, stop=True)
                    nc.scalar.activation(xo_t[:, sl], ps[:], AF.Identity,
                                         bias=vent_b[hb][:, 0:1])
                xo0.append(xo_t)
        x = layernorm(xo0, vlnw, vlnb, "vent")

        # ---------------- mamba layers ----------------
        for l in range(NL):
            u_t = []
            # ---- phase A (u half) + depthwise causal conv + silu ----
            with tc.tile_pool(name=f"Aps{l}", bufs=3, space="PSUM") as aps:
                for d in range(DB):
                    ur = ap.tile([128, PBT], bf16, tag="W", bufs=WBUFS, name=f"uraw{l}_{d}")
                    for b in range(BS):
                        nc.gpsimd.memset(ur[:, b * LP: b * LP + DC - 1], 0.0)
                    for s in range(BS):
                        sl = slice(s * 512, (s + 1) * 512)
                        ps = aps.tile([128, 512], f32, tag="pj", name=f"aps{l}_{d}_{s}")
                        for kb in range(HB):
                            nc.tensor.matmul(ps[:], inwT[l][kb][:, d * 128:(d + 1) * 128],
                                             x[kb][:, sl], start=(kb == 0), stop=(kb == HB - 1))
                        nc.scalar.activation(ur[:, s * LP + DC - 1:(s + 1) * LP], ps[:],
                                             AF.Copy)
                    # conv: full-width shifted 2D slices (stay inside each
                    # 515-seg). ACT does the per-channel scaled shifts (it has
                    # slack); DVE does packed-2x pairwise adds.
                    CW = PBT - DC + 1
                    sh = []
                    for k in range(DC):
                        st = ap.tile([128, PBT], bf16, tag="W", bufs=WBUFS,
                                     name=f"csh{l}_{d}_{k}")
                        nc.vector.tensor_scalar_mul(st[:, 0:CW], ur[:, k:CW + k],
                                                    conv_w[l][d][:, k:k + 1])
                        sh.append(st)
                    a01 = ap.tile([128, PBT], bf16, tag="W", bufs=WBUFS,
                                  name=f"ca01{l}_{d}")
                    nc.vector.tensor_tensor(a01[:, 0:CW], sh[0][:, 0:CW],
                                            sh[1][:, 0:CW], ALU.add)
                    a23 = ap.tile([128, PBT], bf16, tag="W", bufs=WBUFS,
                                  name=f"ca23{l}_{d}")
                    nc.vector.tensor_tensor(a23[:, 0:CW], sh[2][:, 0:CW],
                                            sh[3][:, 0:CW], ALU.add)
                    acc = ap.tile([128, PBT], bf16, tag="W", bufs=WBUFS,
                                  name=f"cacc{l}_{d}")
                    nc.vector.tensor_tensor(acc[:, 0:CW], a01[:, 0:CW],
                                            a23[:, 0:CW], ALU.add)
                    ut = ap.tile([128, BT], bf16, tag="u", bufs=4, name=f"u{l}_{d}")
                    nc.scalar.activation(
                        ut[:].rearrange("p (b t) -> p b t", b=BS),
                        acc[:].rearrange("p (b t) -> p b t", t=LP)[:, :, 0:512],
                        AF.Silu, bias=conv_b[l][d][:, 0:1])
                    u_t.append(ut)
                    # z quarter for this d: fills the PE gap left by conv
                    mb = 4 + d
                    zt = wtile(f"z{l}_{mb}")
                    for s in range(BS):
                        sl = slice(s * 512, (s + 1) * 512)
                        ps = aps.tile([128, 512], f32, tag="pj", name=f"zps{l}_{mb}_{s}")
                        for kb in range(HB):
                            nc.tensor.matmul(ps[:], inwT[l][kb][:, mb * 128:(mb + 1) * 128],
                                             x[kb][:, sl], start=(kb == 0), stop=(kb == HB - 1))
                        nc.vector.tensor_copy(zt[:, sl], ps[:])
                    nc.scalar.dma_start(z_sp.ap()[(mb - 4) * 128:(mb - 3) * 128, :], zt[:])
            # ---- phase C: xproj -> (B | C) rows first (unblocks the E-phase
            # broadcast pipeline), then the dt_in rows ----
            xdb = ap.tile([64, BT], bf16, tag="xd", bufs=1, name=f"xdb{l}")
            xdt = xdb[0:16, :]
            xbc = xdb[32:64, :]
            with tc.tile_pool(name=f"Cps{l}", bufs=3, space="PSUM") as cps:
                for s in range(BS):
                    sl = slice(s * 512, (s + 1) * 512)
                    ps = cps.tile([16, 512], f32, tag="pdt", name=f"cpd{l}_{s}")
                    for kb in range(DB):
                        nc.tensor.matmul(ps[:], xpwT[l][kb][:, 0:16], u_t[kb][:, sl],
                                         start=(kb == 0), stop=(kb == DB - 1))
                    nc.vector.tensor_copy(xdt[:, sl], ps[:])
                for s in range(BS):
                    sl = slice(s * 512, (s + 1) * 512)
                    ps = cps.tile([32, 512], f32, tag="pbc", name=f"cps{l}_{s}")
                    for kb in range(DB):
                        nc.tensor.matmul(ps[:], xpwT[l][kb][:, 16:48], u_t[kb][:, sl],
                                         start=(kb == 0), stop=(kb == DB - 1))
                    nc.vector.tensor_copy(xbc[:, sl], ps[:])
                nc.sync.dma_start(bc_sp.ap()[:, :], xbc[:, :])

            # ---- phase D: dt_proj -> softplus; dtu; poison; spill d>=2 ----
            dt_res, dtu_res = {}, {}
            with tc.tile_pool(name=f"Dps{l}", bufs=3, space="PSUM") as dps:
                for d in range(DB):
                    et = wtile(f"et{l}_{d}")
                    for s in range(BS):
                        sl = slice(s * 512, (s + 1) * 512)
                        ps = dps.tile([128, 512], f32, tag="pj", name=f"dps{l}_{d}_{s}")
                        nc.tensor.matmul(ps[:], dtwT[l][0][0:DR, d * 128:(d + 1) * 128],
                                         xdt[0:DR, sl], start=True, stop=True)
                        # softplus(x+b) = ln(1 + exp(x+b))
                        nc.scalar.activation(et[:, sl], ps[:], AF.Exp,
                                             bias=dt_b[l][d][:, 0:1])
                    if d < 2:
                        dt_t = ap.tile([128, BT], bf16, tag="dt", bufs=2, name=f"dt{l}_{d}")
                    else:
                        dt_t = wtile(f"dtw{l}_{d}")
                    nc.scalar.activation(dt_t[:], et[:], AF.Ln, bias=1.0)
                    if d < 2:
                        dtu = ap.tile([128, BT], bf16, tag="dtu", bufs=2, name=f"dtu{l}_{d}")
                    else:
                        dtu = wtile(f"dtuw{l}_{d}")
                    nc.vector.tensor_tensor(dtu[:], dt_t[:], u_t[d][:], ALU.mult)
                    for b in range(BS):
                        nc.gpsimd.memset(dt_t[:, b * L: b * L + 1], POISON)
                    if d >= 2:
                        nc.scalar.dma_start(dt_sp.ap()[d - 2], dt_t[:])
                        nc.scalar.dma_start(dtu_sp.ap()[d - 2], dtu[:])
                    else:
                        dt_res[d] = dt_t
                        dtu_res[d] = dtu

            # ---- phase E: selective scan ----
            with tc.tile_pool(name=f"Eps{l}", bufs=1, space="PSUM") as eps_pool:
                for d in range(DB):
                    if d < 2:
                        dtL, dtuL = dt_res[d], dtu_res[d]
                    else:
                        dtL = ap.tile([128, BT], bf16, tag="dt", bufs=2, name=f"dtL{l}_{d}")
                        nc.scalar.dma_start(dtL[:], dt_sp.ap()[d - 2])
                        dtuL = ap.tile([128, BT], bf16, tag="dtu", bufs=2, name=f"dtuL{l}_{d}")
                        nc.scalar.dma_start(dtuL[:], dtu_sp.ap()[d - 2])
                    y_ps = eps_pool.tile([128, BT], f32, tag="y", name=f"yps{l}_{d}")
                    # skip term first: y = diag(D) @ u  (so the accumulation
                    # finishes right after the last state's idents)
                    for si in range(BS):
                        sl = slice(si * 512, (si + 1) * 512)
                        nc.tensor.matmul(y_ps[:, sl], D_t[l][d][:], u_t[d][:, sl],
                                         start=True, stop=False)
                    zs = None
                    for n in range(DS):
                        repB = wtile(f"rb{l}_{d}_{n}")
                        nc.sync.dma_start(repB[:], bc_sp.ap()[n, :].partition_broadcast(128))
                        repC = wtile(f"rc{l}_{d}_{n}")
                        nc.gpsimd.dma_start(repC[:],
                                            bc_sp.ap()[DS + n, :].partition_broadcast(128))
                        dA = wtile(f"dA{l}_{d}_{n}")
                        nc.scalar.activation(dA[:], dtL[:], AF.Exp,
                                             scale=A_t[l][d][:, n:n + 1])
                        dBu = wtile(f"dBu{l}_{d}_{n}")
                        nc.vector.tensor_tensor(dBu[:], dtuL[:], repB[:], ALU.mult)
                        h = wtile(f"h{l}_{d}_{n}")
                        nc.vector.tensor_tensor_scan(h[:], dA[:], dBu[:], 0.0,
                                                     ALU.mult, ALU.add)
                        hc = wtile(f"hc{l}_{d}_{n}")
                        nc.vector.tensor_tensor(hc[:], h[:], repC[:], ALU.mult)
                        for si in range(BS):
                            sl = slice(si * 512, (si + 1) * 512)
                            nc.tensor.matmul(y_ps[:, sl], ident[:], hc[:, sl],
                                             start=False, stop=(n == DS - 1))
                        if n == DS - 2:
                            # prefetch + silu the gate input during the last unit
                            zsr = wtile(f"zsr{l}_{d}")
                            nc.sync.dma_start(zsr[:],
                                              z_sp.ap()[d * 128:(d + 1) * 128, :])
                            zs = wtile(f"zs{l}_{d}")
                            nc.scalar.activation(zs[:], zsr[:], AF.Silu)
                    # gate per 512-chunk, pipelined behind the last ident pass
                    for si in range(BS):
                        sl = slice(si * 512, (si + 1) * 512)
                        nc.vector.tensor_tensor(u_t[d][:, sl], zs[:, sl], y_ps[:, sl],
                                                ALU.mult)

            # ---- phase F: out_proj + layernorm ----
            xo = []
            with tc.tile_pool(name=f"Fps{l}", bufs=3, space="PSUM") as fps:
                for hb in range(HB):
                    xo_t = wtile(f"xo{l}_{hb}")
                    for s in range(BS):
                        sl = slice(s * 512, (s + 1) * 512)
                        ps = fps.tile([128, 512], f32, tag="pj", name=f"fps{l}_{hb}_{s}")
                        for kb in range(DB):
                            nc.tensor.matmul(ps[:], outwT[l][kb][:, hb * 128:(hb + 1) * 128],
                                             u_t[kb][:, sl], start=(kb == 0),
                                             stop=(kb == DB - 1))
                        nc.scalar.activation(xo_t[:, sl], ps[:], AF.Copy)
                    xo.append(xo_t)
            x = layernorm(xo, lnw[l], lnb[l], f"l{l}")

        # ---------------- attention pool over time (softmax, no max-sub:
        # logits are O(0.3) so exp is perfectly stable) ----------------
        ex16 = wtile("ex16")  # [1, BT] used on partition 0
        with tc.tile_pool(name="Pps", bufs=3, space="PSUM") as pps:
            for s in range(BS):
                sl = slice(s * 512, (s + 1) * 512)
                ps = pps.tile([1, 512], f32, tag="lgst", name=f"pps{s}")
                for hb in range(HB):
                    nc.tensor.matmul(ps[:], poolT[hb][:, 0:1], x[hb][:, sl],
                                     start=(hb == 0), stop=(hb == HB - 1))
                nc.scalar.activation(ex16[0:1, sl], ps[:], AF.Exp, bias=poolb[0:1, 0:1])
        sm8 = ap.tile([1, BS], f32, tag="smc", name="sm8")
        nc.vector.tensor_reduce(sm8[:], ex16[0:1, :].rearrange("p (b t) -> p b t", b=BS),
                                axis=AX.X, op=ALU.add)
        rs = ap.tile([1, BS], f32, tag="smc", name="rs")
        nc.vector.reciprocal(rs[:], sm8[:])
        nc.sync.dma_start(rs_sp.ap(), rs[:])
        nc.sync.dma_start(ex_sp.ap(), ex16[0:1, :])
        ex_rep = wtile("ex_rep")
        nc.sync.dma_start(ex_rep[:], ex_sp.ap()[0, :].partition_broadcast(128))
        rs_rep = ap.tile([128, BS], f32, tag="rsr", name="rs_rep")
        nc.sync.dma_start(rs_rep[:], rs_sp.ap()[0, :].partition_broadcast(128))
        v_t = []
        for hb in range(HB):
            xa = wtile(f"xa{hb}")
            nc.vector.tensor_tensor(xa[:], x[hb][:], ex_rep[:], ALU.mult)
            vv = ap.tile([128, BS], f32, tag="vsm", bufs=2, name=f"vv{hb}")
            nc.vector.tensor_reduce(vv[:], xa[:].rearrange("p (b t) -> p b t", b=BS),
                                    axis=AX.X, op=ALU.add)
            v16 = ap.tile([128, BS], bf16, tag="vshb", name=f"v16_{hb}")
            nc.vector.tensor_tensor(v16[:], vv[:], rs_rep[:], ALU.mult)
            v_t.append(v16)

        # ---------------- fusion head ----------------
        with tc.tile_pool(name="Hps", bufs=3, space="PSUM") as hps:
            vi = []
            for hb in range(HB):
                t = ap.tile([128, BS], bf16, tag="vit", name=f"vi{hb}")
                nc.vector.tensor_tensor(t[:], v_t[hb][:], ii2[hb][:], ALU.mult)
                vi.append(t)
            f_rhs = [v_t[0], v_t[1], ii2[0], ii2[1], vi[0], vi[1]]
            hh = []
            for mb in range(HB):
                ps = hps.tile([128, BS], f32, tag="hp", name=f"h1p{mb}")
                for kb in range(6):
                    nc.tensor.matmul(ps[:], h1T[kb][:, mb * 128:(mb + 1) * 128],
                                     f_rhs[kb][:], start=(kb == 0), stop=(kb == 5))
                t = ap.tile([128, BS], bf16, tag="hht", name=f"hh{mb}")
                nc.scalar.activation(t[:], ps[:], AF.Relu, bias=hb1[mb][:, 0:1])
                hh.append(t)
            ps = hps.tile([1, BS], f32, tag="hpo", name="outp")
            for kb in range(HB):
                nc.tensor.matmul(ps[:], h2T[kb][:, 0:1], hh[kb][:],
                                 start=(kb == 0), stop=(kb == HB - 1))
            o_sb = ap.tile([1, BS], f32, tag="osb", name="o_sb")
            nc.scalar.activation(o_sb[:], ps[:], AF.Identity, bias=hb2[0:1, 0:1])
        nc.sync.dma_start(out_d.ap(), o_sb[:])

    nc.compile()
    return nc


_NC = None


def _get_nc():
    global _NC
    if _NC is None:
        _NC = _build()
    return _NC


def run(inputs, trace=False):
    nc = _get_nc()
    inputs = {k: np.asarray(v, dtype=np.float32) for k, v in inputs.items()}
    in_maps = []
    for c in range(NCORES):
        m = {name: inputs[name] for name in WEIGHT_NAMES}
        m["xv"] = np.ascontiguousarray(inputs["xv"][c * BS:(c + 1) * BS])
        m["xi"] = np.ascontiguousarray(inputs["xi"][c * BS:(c + 1) * BS])
        in_maps.append(m)
    res = run_bass_kernel_spmd(nc, in_maps, core_ids=list(range(NCORES)), trace=trace)
    out = np.concatenate([np.asarray(res.results[c]["out"]).reshape(BS)
                          for c in range(NCORES)])
    return out.reshape(B, 1).astype(np.float32), res.exec_time_ns


def kernel(**inputs):
    return run(inputs, trace=False)[0]


# revision 48
# speedup vs baseline: 1.0726x; 1.0132x over previous
"""Trainium2 Bass kernel for nn_CrossFusionMamba (2-layer Mamba stack + fusion head).

Self-contained: hardcodes all shapes/sharding. Data-parallel over batch across
8 NeuronCores (8 batch elements per core).

Layout: channels on SBUF partitions, flattened (batch, time) on the free dim
(bt = b*512 + t -> 4096 columns per core). Full-BT tiles everywhere.

Engine assignment for the selective scan (the bottleneck):
  ACT   : dA = exp(A[d,n] * dt)            (per-partition scale)
  DVE   : dBu = (dt*u) . B_n ; h = tensor_tensor_scan(dA, dBu) ; hc = h . C_n
  PE    : y = diag(D) @ u + sum_n I @ hc_n (PSUM accumulation, skip term first)
GpSimd does NO compute: running Pool tensor_tensor concurrently with DVE
tensor_tensor degrades BOTH ~2-4x (SBUF port contention) — measured, twice.
DVE tensor_tensor in packed bf16 SBUF hits the 2x mode (~0.56 ns/col); the
scan runs at ~2.1 ns/col + ~2us fixed, which is the hard floor here.
Batch independence inside one scan op is enforced by poisoning dt at each
batch's first column (dt=1e9 -> dA=exp(-big)=0 -> exact state reset).

B/C rows are spilled to DRAM once per layer and partition-broadcast to
[128, BT] tiles per (d, n) (SP queue for B, gpsimd SWDGE for C).
Weights are loaded via gpsimd casting DMAs (f32->bf16) and PE transposes.
Big transients share one 9-deep rotating slot tag ("W"); slot-reuse WAR
edges are safe because every W tile's readers are emitted within 9
subsequent W allocations.
"""
import sys

if "/opt/trn_rl_repo" not in sys.path:
    sys.path.insert(0, "/opt/trn_rl_repo")

from contextlib import ExitStack

import numpy as np

import concourse.bacc as bacc
import concourse.tile as tile
import concourse.mybir as mybir
from concourse.bass_utils import run_bass_kernel_spmd

f32 = mybir.dt.float32
bf16 = mybir.dt.bfloat16
AF = mybir.ActivationFunctionType
ALU = mybir.AluOpType
AX = mybir.AxisListType

# model dims
B, L, VD, ID = 64, 512, 64, 32
H, DI, DS, DC, DR, NL = 256, 512, 16, 4, 16, 2
NCORES = 8
BS = B // NCORES          # batches per core
BT = BS * L               # free columns per core (4096)
LP = L + DC - 1           # padded per-batch length for conv (515)
PBT = BS * LP             # 4120
HB = H // 128             # 2
DB = DI // 128            # 4
POISON = 1.0e9


WEIGHT_NAMES = [
    "vent_in_w", "vent_in_b", "vent_ln_w", "vent_ln_b",
    "m_in_w", "m_conv_w", "m_conv_b", "m_xproj_w", "m_dt_w", "m_dt_b",
    "m_Alog", "m_D", "m_out_w", "m_ln_w", "m_ln_b",
    "pool_w", "pool_b", "img_w1", "img_b1", "img_w2", "img_b2",
    "head_w1", "head_b1", "head_w2", "head_b2",
]


def _build():
    nc = bacc.Bacc("TRN2", target_bir_lowering=False, debug=False)

    # ---- DRAM I/O ----
    xv_d = nc.dram_tensor("xv", [BS, L, VD], f32, kind="ExternalInput")
    xi_d = nc.dram_tensor("xi", [BS, ID], f32, kind="ExternalInput")
    wd = {}
    for name, shape in [
        ("vent_in_w", [H, VD]), ("vent_in_b", [H]), ("vent_ln_w", [H]), ("vent_ln_b", [H]),
        ("m_in_w", [NL, 2 * DI, H]), ("m_conv_w", [NL, DI, DC]), ("m_conv_b", [NL, DI]),
        ("m_xproj_w", [NL, DR + 2 * DS, DI]), ("m_dt_w", [NL, DI, DR]), ("m_dt_b", [NL, DI]),
        ("m_Alog", [NL, DI, DS]), ("m_D", [NL, DI]), ("m_out_w", [NL, H, DI]),
        ("m_ln_w", [NL, H]), ("m_ln_b", [NL, H]),
        ("pool_w", [1, H]), ("pool_b", [1]),
        ("img_w1", [H, ID]), ("img_b1", [H]), ("img_w2", [H, H]), ("img_b2", [H]),
        ("head_w1", [H, 3 * H]), ("head_b1", [H]), ("head_w2", [1, H]), ("head_b2", [1]),
    ]:
        wd[name] = nc.dram_tensor(name, shape, f32, kind="ExternalInput")
    out_d = nc.dram_tensor("out", [1, BS], f32, kind="ExternalOutput")

    # DRAM scratch
    bc_sp = nc.dram_tensor("bc_sp", [2 * DS, BT], bf16)   # B rows 0:16, C rows 16:32
    z_sp = nc.dram_tensor("z_sp", [DI, BT], bf16)         # silu(z) spill
    dt_sp = nc.dram_tensor("dt_sp", [2, 128, BT], bf16)   # dt spill for d=2,3
    dtu_sp = nc.dram_tensor("dtu_sp", [2, 128, BT], bf16)
    st_sp = nc.dram_tensor("st_sp", [2, BT], bf16)        # LN mu/inv bf16 rows
    ex_sp = nc.dram_tensor("ex_sp", [1, BT], bf16)        # attn-pool exp row
    rs_sp = nc.dram_tensor("rs_sp", [1, BS], f32)         # attn-pool 1/sum

    with tile.TileContext(nc) as tc, ExitStack() as ctx:
        wpool = ctx.enter_context(tc.tile_pool(name="wpool", bufs=1))
        ap = ctx.enter_context(tc.tile_pool(name="ap", bufs=2))

        WBUFS = 9

        def wtile(name):
            """Big rotating transient slot [128, <=4120]."""
            return ap.tile([128, BT], bf16, tag="W", bufs=WBUFS, name=name)

        # ---------------- constants ----------------
        ident = wpool.tile([128, 128], bf16, name="ident")
        nc.vector.memset(ident[:], 1.0)
        nc.gpsimd.affine_select(ident[:], ident[:], pattern=[[-1, 128]], base=0,
                                channel_multiplier=1, compare_op=ALU.is_equal, fill=0.0)
        ones_col = wpool.tile([128, 1], bf16, name="ones_col")
        nc.vector.memset(ones_col[:], 1.0)
        eps_col = wpool.tile([128, 1], f32, name="eps_col")
        nc.vector.memset(eps_col[:], 1e-5)

        # ---------------- input DMAs first (own the SP queue head) ----------
        xiT = ap.tile([ID, BS], f32, tag="xiT", name="xiT")
        nc.sync.dma_start(xiT[:], xi_d.ap().rearrange("b f -> f b"))

        xv16 = ap.tile([128, BS * L * VD // 128], bf16, tag="W", bufs=WBUFS, name="xv16")
        xv_3d = xv_d.ap().rearrange("b l v -> (b l) v").rearrange("(c p) v -> p c v", p=128)
        nc.gpsimd.dma_start(
            xv16[:].rearrange("p (c v) -> p c v", v=VD), xv_3d)

        # ---------------- weight preprocessing ----------------
        ld_ctx = ExitStack()
        ldp = ld_ctx.enter_context(tc.tile_pool(name="ldp", bufs=3))
        ldps = ld_ctx.enter_context(tc.tile_pool(name="ldps", bufs=2, space="PSUM"))

        def load_cols(src_ap, n, name):
            """1-D DRAM vector [n] -> list of [128,1] f32 col tiles."""
            cols = []
            for blk in range((n + 127) // 128):
                m = min(128, n - blk * 128)
                t = wpool.tile([m, 1], f32, name=f"{name}_c{blk}")
                nc.sync.dma_start(t[:, 0:1],
                                  src_ap[blk * 128: blk * 128 + m].rearrange("(a b) -> a b", b=1))
                cols.append(t)
            return cols

        def load_T(src_ap, R, C, name):
            """DRAM [R, C] f32 -> transposed bf16 SBUF tiles (list over C-blocks of [cm, R]).

            Casting f32->bf16 happens inside the gpsimd software-DGE DMA."""
            nrb = (R + 127) // 128
            ncb = (C + 127) // 128
            outs = []
            for cb in range(ncb):
                cm = min(128, C - cb * 128)
                t = wpool.tile([cm, R], bf16, name=f"{name}_T{cb}")
                outs.append(t)
            for rb in range(nrb):
                rm = min(128, R - rb * 128)
                nat16 = ldp.tile([rm, C], bf16, tag="ld16", name=f"{name}_m{rb}")
                nc.gpsimd.dma_start(nat16[:], src_ap[rb * 128: rb * 128 + rm, :])
                for cb in range(ncb):
                    cm = min(128, C - cb * 128)
                    tp = ldps.tile([cm, rm], bf16, tag="ldT", name=f"{name}_p{rb}_{cb}")
                    nc.tensor.transpose(tp[:], nat16[:, cb * 128: cb * 128 + cm],
                                        ident[0:rm, 0:rm])
                    nc.vector.tensor_copy(outs[cb][:, rb * 128: rb * 128 + rm], tp[:])
            return outs

        ventT = load_T(wd["vent_in_w"].ap(), H, VD, "ventT")          # 1 x [64, 256]
        vent_b = load_cols(wd["vent_in_b"].ap(), H, "vent_b")
        vlnw = load_cols(wd["vent_ln_w"].ap(), H, "vlnw")
        vlnb = load_cols(wd["vent_ln_b"].ap(), H, "vlnb")

        # ---------------- layernorm over H (partition dim) ----------------
        def layernorm(xo, w_cols, b_cols, tag):
            """xo: HB bf16 [128, BT] tiles (pre-norm) -> normalized tiles (tag 'x')."""
            sq = []
            for hb in range(HB):
                sqt = wtile(f"sq_{tag}_{hb}")
                nc.vector.tensor_tensor(sqt[:], xo[hb][:], xo[hb][:], ALU.mult)
                sq.append(sqt)
            mu8 = ap.tile([BS, 512], f32, tag="ln8", bufs=4, name=f"mu8_{tag}")
            msq8 = ap.tile([BS, 512], f32, tag="ln8", bufs=4, name=f"msq8_{tag}")
            with tc.tile_pool(name=f"lnps_{tag}", bufs=2, space="PSUM") as lps:
                for s in range(BS):
                    sl = slice(s * 512, (s + 1) * 512)
                    ps_x = lps.tile([1, 512], f32, tag="lnst1", name=f"sx_{tag}_{s}")
                    for hb in range(HB):
                        nc.tensor.matmul(ps_x[:], ones_col[:], xo[hb][:, sl],
                                         start=(hb == 0), stop=(hb == HB - 1))
                    sxs = ap.tile([1, 512], f32, tag="lnsl", bufs=2, name=f"sxs_{tag}_{s}")
                    nc.scalar.activation(sxs[:], ps_x[:], AF.Copy, scale=1.0 / H)
                    nc.sync.dma_start(mu8[s:s + 1, :], sxs[:])
                    ps_q = lps.tile([1, 512], f32, tag="lnst2", name=f"sq_{tag}_{s}")
                    for hb in range(HB):
                        nc.tensor.matmul(ps_q[:], ones_col[:], sq[hb][:, sl],
                                         start=(hb == 0), stop=(hb == HB - 1))
                    sqs2 = ap.tile([1, 512], f32, tag="lnsl", bufs=2, name=f"sqs_{tag}_{s}")
                    nc.scalar.activation(sqs2[:], ps_q[:], AF.Copy, scale=1.0 / H)
                    nc.sync.dma_start(msq8[s:s + 1, :], sqs2[:])
            var8 = ap.tile([BS, 512], f32, tag="ln8", bufs=4, name=f"var8_{tag}")
            nc.vector.tensor_tensor(var8[:], mu8[:], mu8[:], ALU.mult)
            nc.vector.tensor_tensor(var8[:], msq8[:], var8[:], ALU.subtract)
            sd8 = ap.tile([BS, 512], f32, tag="ln8", bufs=4, name=f"sd8_{tag}")
            nc.scalar.activation(sd8[:], var8[:], AF.Sqrt, bias=eps_col[0:BS, 0:1])
            inv8 = ap.tile([BS, 512], f32, tag="ln8", bufs=4, name=f"inv8_{tag}")
            nc.vector.reciprocal(inv8[:], sd8[:])
            mu16 = ap.tile([BS, 512], bf16, tag="ln16", bufs=2, name=f"mu16_{tag}")
            nc.vector.tensor_copy(mu16[:], mu8[:])
            inv16 = ap.tile([BS, 512], bf16, tag="ln16", bufs=2, name=f"inv16_{tag}")
            nc.vector.tensor_copy(inv16[:], inv8[:])
            nc.sync.dma_start(st_sp.ap()[0, :].rearrange("(b t) -> b t", b=BS), mu16[:])
            nc.sync.dma_start(st_sp.ap()[1, :].rearrange("(b t) -> b t", b=BS), inv16[:])
            mu_rep = wtile(f"murep_{tag}")
            nc.sync.dma_start(mu_rep[:], st_sp.ap()[0, :].partition_broadcast(128))
            inv_rep = wtile(f"invrep_{tag}")
            nc.scalar.dma_start(inv_rep[:], st_sp.ap()[1, :].partition_broadcast(128))
            xcs = []
            for hb in range(HB):
                xc = wtile(f"xc_{tag}_{hb}")
                nc.vector.tensor_tensor(xc[:], xo[hb][:], mu_rep[:], ALU.subtract)
                xcs.append(xc)
            x_out = []
            for hb in range(HB):
                xn = wtile(f"xn_{tag}_{hb}")
                nc.vector.tensor_tensor(xn[:], xcs[hb][:], inv_rep[:], ALU.mult)
                xt = ap.tile([128, BT], bf16, tag="x", bufs=2, name=f"x_{tag}_{hb}")
                nc.scalar.activation(xt[:], xn[:], AF.Identity,
                                     scale=w_cols[hb][:, 0:1], bias=b_cols[hb][:, 0:1])
                x_out.append(xt)
            return x_out

        # ---------------- vent input projection ----------------
        xvT = wtile("xvT")  # [64, BT] on first 64 partitions
        xo0 = []
        with tc.tile_pool(name="xvps", bufs=3, space="PSUM") as xps, \
             tc.tile_pool(name="ventps", bufs=3, space="PSUM") as vps:
            for blk in range(BT // 128):
                tp = xps.tile([VD, 128], bf16, tag="xvT", name=f"xvp{blk}")
                nc.tensor.transpose(tp[:], xv16[:, blk * VD:(blk + 1) * VD], ident[:])
                nc.vector.tensor_copy(xvT[0:VD, blk * 128:(blk + 1) * 128], tp[:])
            for hb in range(HB):
                xo_t = wtile(f"vxo{hb}")
                for s in range(BS):
                    sl = slice(s * 512, (s + 1) * 512)
                    ps = vps.tile([128, 512], f32, tag="pj", name=f"vps{hb}_{s}")
                    nc.tensor.matmul(ps[:], ventT[0][:, hb * 128:(hb + 1) * 128],
                                     xvT[0:VD, sl], start=True, stop=True)
                    nc.scalar.activation(xo_t[:, sl], ps[:], AF.Identity,
                                         bias=vent_b[hb][:, 0:1])
                xo0.append(xo_t)
        x = layernorm(xo0, vlnw, vlnb, "vent")

        inwT, xpwT, dtwT, outwT = [], [], [], []
        conv_w, conv_b, dt_b, A_t, D_t, lnw, lnb = [], [], [], [], [], [], []
        for l in range(NL):
            inwT.append(load_T(wd["m_in_w"].ap()[l], 2 * DI, H, f"inwT{l}"))
            xpwT.append(load_T(wd["m_xproj_w"].ap()[l], DR + 2 * DS, DI, f"xpwT{l}"))
            dtwT.append(load_T(wd["m_dt_w"].ap()[l], DI, DR, f"dtwT{l}"))
            outwT.append(load_T(wd["m_out_w"].ap()[l], H, DI, f"outwT{l}"))
            cwl, al = [], []
            for d in range(DB):
                sl = slice(d * 128, (d + 1) * 128)
                cw = wpool.tile([128, DC], f32, name=f"cw{l}_{d}")
                nc.sync.dma_start(cw[:], wd["m_conv_w"].ap()[l, sl, :])
                cwl.append(cw)
                alog = ldp.tile([128, DS], f32, tag="alog", name=f"alog{l}_{d}")
                nc.sync.dma_start(alog[:], wd["m_Alog"].ap()[l, sl, :])
                a = wpool.tile([128, DS], f32, name=f"A{l}_{d}")
                nc.scalar.activation(a[:], alog[:], AF.Exp)
                nc.vector.tensor_scalar_mul(a[:], a[:], -1.0)
                al.append(a)
            conv_w.append(cwl)
            conv_b.append(load_cols(wd["m_conv_b"].ap()[l], DI, f"cb{l}"))
            dt_b.append(load_cols(wd["m_dt_b"].ap()[l], DI, f"dtb{l}"))
            Dcols = load_cols(wd["m_D"].ap()[l], DI, f"D{l}")
            dgl = []
            for d in range(DB):
                dg = wpool.tile([128, 128], bf16, name=f"dg{l}_{d}")
                nc.vector.tensor_scalar_mul(dg[:], ident[:], Dcols[d][:, 0:1])
                dgl.append(dg)
            D_t.append(dgl)
            A_t.append(al)
            lnw.append(load_cols(wd["m_ln_w"].ap()[l], H, f"lnw{l}"))
            lnb.append(load_cols(wd["m_ln_b"].ap()[l], H, f"lnb{l}"))
        poolT = load_T(wd["pool_w"].ap(), 1, H, "poolT")              # 2 x [128, 1]
        poolb = wpool.tile([1, 1], f32, name="poolb")
        nc.sync.dma_start(poolb[:], wd["pool_b"].ap().rearrange("(a b) -> a b", b=1))
        imgw1T = load_T(wd["img_w1"].ap(), H, ID, "imgw1T")           # 1 x [32, 256]
        imgb1 = load_cols(wd["img_b1"].ap(), H, "imgb1")
        imgw2T = load_T(wd["img_w2"].ap(), H, H, "imgw2T")            # 2 x [128, 256]
        imgb2 = load_cols(wd["img_b2"].ap(), H, "imgb2")
        h1T = load_T(wd["head_w1"].ap(), H, 3 * H, "h1T")             # 6 x [128, 256]
        hb1 = load_cols(wd["head_b1"].ap(), H, "hb1")
        h2T = load_T(wd["head_w2"].ap(), 1, H, "h2T")                 # 2 x [128, 1]
        hb2 = wpool.tile([1, 1], f32, name="hb2")
        nc.sync.dma_start(hb2[:], wd["head_b2"].ap().rearrange("(a b) -> a b", b=1))
        ld_ctx.close()
        # ---------------- image branch (independent of the mamba stack) -----
        xiT16 = ap.tile([ID, BS], bf16, tag="xiT16", name="xiT16")
        nc.vector.tensor_copy(xiT16[:], xiT[:])
        ii2 = []
        with tc.tile_pool(name="Ips", bufs=2, space="PSUM") as ips:
            ii1 = []
            for hb in range(HB):
                ps = ips.tile([128, BS], f32, tag="hp", name=f"i1p{hb}")
                nc.tensor.matmul(ps[:], imgw1T[0][0:ID, hb * 128:(hb + 1) * 128], xiT16[:],
                                 start=True, stop=True)
                t = ap.tile([128, BS], bf16, tag="ii1t", name=f"ii1_{hb}")
                nc.scalar.activation(t[:], ps[:], AF.Relu, bias=imgb1[hb][:, 0:1])
                ii1.append(t)
            for hb in range(HB):
                ps = ips.tile([128, BS], f32, tag="hp", name=f"i2p{hb}")
                for kb in range(HB):
                    nc.tensor.matmul(ps[:], imgw2T[kb][:, hb * 128:(hb + 1) * 128],
                                     ii1[kb][:], start=(kb == 0), stop=(kb == HB - 1))
                t = ap.tile([128, BS], bf16, tag="ii2t", name=f"ii2_{hb}")
                nc.scalar.activation(t[:], ps[:], AF.Relu, bias=imgb2[hb][:, 0:1])
                ii2.append(t)

        # ---------------- mamba layers ----------------
        for l in range(NL):
            u_t = []
            # ---- phase A (u half) + depthwise causal conv + silu ----
            with tc.tile_pool(name=f"Aps{l}", bufs=3, space="PSUM") as aps:
                for d in range(DB):
                    ur = ap.tile([128, PBT], bf16, tag="W", bufs=WBUFS, name=f"uraw{l}_{d}")
                    for b in range(BS):
                        nc.gpsimd.memset(ur[:, b * LP: b * LP + DC - 1], 0.0)
                    for s in range(BS):
                        sl = slice(s * 512, (s + 1) * 512)
                        ps = aps.tile([128, 512], f32, tag="pj", name=f"aps{l}_{d}_{s}")
                        for kb in range(HB):
                            nc.tensor.matmul(ps[:], inwT[l][kb][:, d * 128:(d + 1) * 128],
                                             x[kb][:, sl], start=(kb == 0), stop=(kb == HB - 1))
                        nc.scalar.activation(ur[:, s * LP + DC - 1:(s + 1) * LP], ps[:],
                                             AF.Copy)
                    # conv: full-width shifted 2D slices (stay inside each
                    # 515-seg). ACT does the per-channel scaled shifts (it has
                    # slack); DVE does packed-2x pairwise adds.
                    CW = PBT - DC + 1
                    sh = []
                    for k in range(DC):
                        st = ap.tile([128, PBT], bf16, tag="W", bufs=WBUFS,
                                     name=f"csh{l}_{d}_{k}")
                        nc.vector.tensor_scalar_mul(st[:, 0:CW], ur[:, k:CW + k],
                                                    conv_w[l][d][:, k:k + 1])
                        sh.append(st)
                    a01 = ap.tile([128, PBT], bf16, tag="W", bufs=WBUFS,
                                  name=f"ca01{l}_{d}")
                    nc.vector.tensor_tensor(a01[:, 0:CW], sh[0][:, 0:CW],
                                            sh[1][:, 0:CW], ALU.add)
                    a23 = ap.tile([128, PBT], bf16, tag="W", bufs=WBUFS,
                                  name=f"ca23{l}_{d}")
                    nc.vector.tensor_tensor(a23[:, 0:CW], sh[2][:, 0:CW],
                                            sh[3][:, 0:CW], ALU.add)
                    acc = ap.tile([128, PBT], bf16, tag="W", bufs=WBUFS,
                                  name=f"cacc{l}_{d}")
                    nc.vector.tensor_tensor(acc[:, 0:CW], a01[:, 0:CW],
                                            a23[:, 0:CW], ALU.add)
                    ut = ap.tile([128, BT], bf16, tag="u", bufs=4, name=f"u{l}_{d}")
                    nc.scalar.activation(
                        ut[:].rearrange("p (b t) -> p b t", b=BS),
                        acc[:].rearrange("p (b t) -> p b t", t=LP)[:, :, 0:512],
                        AF.Silu, bias=conv_b[l][d][:, 0:1])
                    u_t.append(ut)
                    # z quarter for this d: fills the PE gap left by conv
                    mb = 4 + d
                    zt = wtile(f"z{l}_{mb}")
                    for s in range(BS):
                        sl = slice(s * 512, (s + 1) * 512)
                        ps = aps.tile([128, 512], f32, tag="pj", name=f"zps{l}_{mb}_{s}")
                        for kb in range(HB):
                            nc.tensor.matmul(ps[:], inwT[l][kb][:, mb * 128:(mb + 1) * 128],
                                             x[kb][:, sl], start=(kb == 0), stop=(kb == HB - 1))
                        nc.vector.tensor_copy(zt[:, sl], ps[:])
                    nc.scalar.dma_start(z_sp.ap()[(mb - 4) * 128:(mb - 3) * 128, :], zt[:])
            # ---- phase C: xproj -> (B | C) rows first (unblocks the E-phase
            # broadcast pipeline), then the dt_in rows ----
            xdb = ap.tile([64, BT], bf16, tag="xd", bufs=1, name=f"xdb{l}")
            xdt = xdb[0:16, :]
            xbc = xdb[32:64, :]
            with tc.tile_pool(name=f"Cps{l}", bufs=3, space="PSUM") as cps:
                for s in range(BS):
                    sl = slice(s * 512, (s + 1) * 512)
                    ps = cps.tile([16, 512], f32, tag="pdt", name=f"cpd{l}_{s}")
                    for kb in range(DB):
                        nc.tensor.matmul(ps[:], xpwT[l][kb][:, 0:16], u_t[kb][:, sl],
                                         start=(kb == 0), stop=(kb == DB - 1))
                    nc.vector.tensor_copy(xdt[:, sl], ps[:])
                for s in range(BS):
                    sl = slice(s * 512, (s + 1) * 512)
                    ps = cps.tile([32, 512], f32, tag="pbc", name=f"cps{l}_{s}")
                    for kb in range(DB):
                        nc.tensor.matmul(ps[:], xpwT[l][kb][:, 16:48], u_t[kb][:, sl],
                                         start=(kb == 0), stop=(kb == DB - 1))
                    nc.vector.tensor_copy(xbc[:, sl], ps[:])
                nc.sync.dma_start(bc_sp.ap()[:, :], xbc[:, :])

            # ---- phase D: dt_proj -> softplus; dtu; poison; spill d>=2 ----
            dt_res, dtu_res = {}, {}
            with tc.tile_pool(name=f"Dps{l}", bufs=3, space="PSUM") as dps:
                for d in range(DB):
                    et = wtile(f"et{l}_{d}")
                    for s in range(BS):
                        sl = slice(s * 512, (s + 1) * 512)
                        ps = dps.tile([128, 512], f32, tag="pj", name=f"dps{l}_{d}_{s}")
                        nc.tensor.matmul(ps[:], dtwT[l][0][0:DR, d * 128:(d + 1) * 128],
                                         xdt[0:DR, sl], start=True, stop=True)
                        # softplus(x+b) = ln(1 + exp(x+b))
                        nc.scalar.activation(et[:, sl], ps[:], AF.Exp,
                                             bias=dt_b[l][d][:, 0:1])
                    if d < 2:
                        dt_t = ap.tile([128, BT], bf16, tag="dt", bufs=2, name=f"dt{l}_{d}")
                    else:
                        dt_t = wtile(f"dtw{l}_{d}")
                    nc.scalar.activation(dt_t[:], et[:], AF.Ln, bias=1.0)
                    if d < 2:
                        dtu = ap.tile([128, BT], bf16, tag="dtu", bufs=2, name=f"dtu{l}_{d}")
                    else:
                        dtu = wtile(f"dtuw{l}_{d}")
                    nc.vector.tensor_tensor(dtu[:], dt_t[:], u_t[d][:], ALU.mult)
                    for b in range(BS):
                        nc.gpsimd.memset(dt_t[:, b * L: b * L + 1], POISON)
                    if d >= 2:
                        nc.scalar.dma_start(dt_sp.ap()[d - 2], dt_t[:])
                        nc.scalar.dma_start(dtu_sp.ap()[d - 2], dtu[:])
                    else:
                        dt_res[d] = dt_t
                        dtu_res[d] = dtu

            # ---- phase E: selective scan ----
            with tc.tile_pool(name=f"Eps{l}", bufs=1, space="PSUM") as eps_pool:
                for d in range(DB):
                    if d < 2:
                        dtL, dtuL = dt_res[d], dtu_res[d]
                    else:
                        dtL = ap.tile([128, BT], bf16, tag="dt", bufs=2, name=f"dtL{l}_{d}")
                        nc.scalar.dma_start(dtL[:], dt_sp.ap()[d - 2])
                        dtuL = ap.tile([128, BT], bf16, tag="dtu", bufs=2, name=f"dtuL{l}_{d}")
                        nc.scalar.dma_start(dtuL[:], dtu_sp.ap()[d - 2])
                    y_ps = eps_pool.tile([128, BT], f32, tag="y", name=f"yps{l}_{d}")
                    # skip term first: y = diag(D) @ u  (so the accumulation
                    # finishes right after the last state's idents)
                    for si in range(BS):
                        sl = slice(si * 512, (si + 1) * 512)
                        nc.tensor.matmul(y_ps[:, sl], D_t[l][d][:], u_t[d][:, sl],
                                         start=True, stop=False)
                    zs = None
                    for n in range(DS):
                        repB = wtile(f"rb{l}_{d}_{n}")
                        nc.sync.dma_start(repB[:], bc_sp.ap()[n, :].partition_broadcast(128))
                        repC = wtile(f"rc{l}_{d}_{n}")
                        nc.gpsimd.dma_start(repC[:],
                                            bc_sp.ap()[DS + n, :].partition_broadcast(128))
                        dA = wtile(f"dA{l}_{d}_{n}")
                        nc.scalar.activation(dA[:], dtL[:], AF.Exp,
                                             scale=A_t[l][d][:, n:n + 1])
                        dBu = wtile(f"dBu{l}_{d}_{n}")
                        nc.vector.tensor_tensor(dBu[:], dtuL[:], repB[:], ALU.mult)
                        h = wtile(f"h{l}_{d}_{n}")
                        nc.vector.tensor_tensor_scan(h[:], dA[:], dBu[:], 0.0,
                                                     ALU.mult, ALU.add)
                        hc = wtile(f"hc{l}_{d}_{n}")
                        nc.vector.tensor_tensor(hc[:], h[:], repC[:], ALU.mult)
                        for si in range(BS):
                            sl = slice(si * 512, (si + 1) * 512)
                            nc.tensor.matmul(y_ps[:, sl], ident[:], hc[:, sl],
                                             start=False, stop=(n == DS - 1))
                        if n == DS - 2:
                            # prefetch + silu the gate input during the last unit
                            zsr = wtile(f"zsr{l}_{d}")
                            nc.sync.dma_start(zsr[:],
                                              z_sp.ap()[d * 128:(d + 1) * 128, :])
                            zs = wtile(f"zs{l}_{d}")
                            nc.scalar.activation(zs[:], zsr[:], AF.Silu)
                    # gate per 512-chunk, pipelined behind the last ident pass
                    for si in range(BS):
                        sl = slice(si * 512, (si + 1) * 512)
                        nc.vector.tensor_tensor(u_t[d][:, sl], zs[:, sl], y_ps[:, sl],
                                                ALU.mult)

            # ---- phase F: out_proj + layernorm ----
            xo = []
            with tc.tile_pool(name=f"Fps{l}", bufs=3, space="PSUM") as fps:
                for hb in range(HB):
                    xo_t = wtile(f"xo{l}_{hb}")
                    for s in range(BS):
                        sl = slice(s * 512, (s + 1) * 512)
                        ps = fps.tile([128, 512], f32, tag="pj", name=f"fps{l}_{hb}_{s}")
                        for kb in range(DB):
                            nc.tensor.matmul(ps[:], outwT[l][kb][:, hb * 128:(hb + 1) * 128],
                                             u_t[kb][:, sl], start=(kb == 0),
                                             stop=(kb == DB - 1))
                        nc.scalar.activation(xo_t[:, sl], ps[:], AF.Copy)
                    xo.append(xo_t)
            x = layernorm(xo, lnw[l], lnb[l], f"l{l}")

        # ---------------- attention pool over time (softmax, no max-sub:
        # logits are O(0.3) so exp is perfectly stable) ----------------
        ex16 = wtile("ex16")  # [1, BT] used on partition 0
        with tc.tile_pool(name="Pps", bufs=3, space="PSUM") as pps:
            for s in range(BS):
                sl = slice(s * 512, (s + 1) * 512)
                ps = pps.tile([1, 512], f32, tag="lgst", name=f"pps{s}")
                for hb in range(HB):
                    nc.tensor.matmul(ps[:], poolT[hb][:, 0:1], x[hb][:, sl],
                                     start=(hb == 0), stop=(hb == HB - 1))
                nc.scalar.activation(ex16[0:1, sl], ps[:], AF.Exp, bias=poolb[0:1, 0:1])
        sm8 = ap.tile([1, BS], f32, tag="smc", name="sm8")
        nc.vector.tensor_reduce(sm8[:], ex16[0:1, :].rearrange("p (b t) -> p b t", b=BS),
                                axis=AX.X, op=ALU.add)
        rs = ap.tile([1, BS], f32, tag="smc", name="rs")
        nc.vector.reciprocal(rs[:], sm8[:])
        nc.sync.dma_start(rs_sp.ap(), rs[:])
        nc.sync.dma_start(ex_sp.ap(), ex16[0:1, :])
        ex_rep = wtile("ex_rep")
        nc.sync.dma_start(ex_rep[:], ex_sp.ap()[0, :].partition_broadcast(128))
        rs_rep = ap.tile([128, BS], f32, tag="rsr", name="rs_rep")
        nc.sync.dma_start(rs_rep[:], rs_sp.ap()[0, :].partition_broadcast(128))
        v_t = []
        for hb in range(HB):
            xa = wtile(f"xa{hb}")
            nc.vector.tensor_tensor(xa[:], x[hb][:], ex_rep[:], ALU.mult)
            vv = ap.tile([128, BS], f32, tag="vsm", bufs=2, name=f"vv{hb}")
            nc.vector.tensor_reduce(vv[:], xa[:].rearrange("p (b t) -> p b t", b=BS),
                                    axis=AX.X, op=ALU.add)
            v16 = ap.tile([128, BS], bf16, tag="vshb", name=f"v16_{hb}")
            nc.vector.tensor_tensor(v16[:], vv[:], rs_rep[:], ALU.mult)
            v_t.append(v16)

        # ---------------- fusion head ----------------
        with tc.tile_pool(name="Hps", bufs=3, space="PSUM") as hps:
            vi = []
            for hb in range(HB):
                t = ap.tile([128, BS], bf16, tag="vit", name=f"vi{hb}")
                nc.vector.tensor_tensor(t[:], v_t[hb][:], ii2[hb][:], ALU.mult)
                vi.append(t)
            f_rhs = [v_t[0], v_t[1], ii2[0], ii2[1], vi[0], vi[1]]
            hh = []
            for mb in range(HB):
                ps = hps.tile([128, BS], f32, tag="hp", name=f"h1p{mb}")
                for kb in range(6):
                    nc.tensor.matmul(ps[:], h1T[kb][:, mb * 128:(mb + 1) * 128],
                                     f_rhs[kb][:], start=(kb == 0), stop=(kb == 5))
                t = ap.tile([128, BS], bf16, tag="hht", name=f"hh{mb}")
                nc.scalar.activation(t[:], ps[:], AF.Relu, bias=hb1[mb][:, 0:1])
                hh.append(t)
            ps = hps.tile([1, BS], f32, tag="hpo", name="outp")
            for kb in range(HB):
                nc.tensor.matmul(ps[:], h2T[kb][:, 0:1], hh[kb][:],
                                 start=(kb == 0), stop=(kb == HB - 1))
            o_sb = ap.tile([1, BS], f32, tag="osb", name="o_sb")
            nc.scalar.activation(o_sb[:], ps[:], AF.Identity, bias=hb2[0:1, 0:1])
        nc.sync.dma_start(out_d.ap(), o_sb[:])

    nc.compile()
    return nc


_NC = None


def _get_nc():
    global _NC
    if _NC is None:
        _NC = _build()
    return _NC


def run(inputs, trace=False):
    nc = _get_nc()
    inputs = {k: np.asarray(v, dtype=np.float32) for k, v in inputs.items()}
    in_maps = []
    for c in range(NCORES):
        m = {name: inputs[name] for name in WEIGHT_NAMES}
        m["xv"] = np.ascontiguousarray(inputs["xv"][c * BS:(c + 1) * BS])
        m["xi"] = np.ascontiguousarray(inputs["xi"][c * BS:(c + 1) * BS])
        in_maps.append(m)
    res = run_bass_kernel_spmd(nc, in_maps, core_ids=list(range(NCORES)), trace=trace)
    out = np.concatenate([np.asarray(res.results[c]["out"]).reshape(BS)
                          for c in range(NCORES)])
    return out.reshape(B, 1).astype(np.float32), res.exec_time_ns


def kernel(**inputs):
    return run(inputs, trace=False)[0]


# revision 50
# speedup vs baseline: 1.0739x; 1.0012x over previous
"""Trainium2 Bass kernel for nn_CrossFusionMamba (2-layer Mamba stack + fusion head).

Self-contained: hardcodes all shapes/sharding. Data-parallel over batch across
8 NeuronCores (8 batch elements per core).

Layout: channels on SBUF partitions, flattened (batch, time) on the free dim
(bt = b*512 + t -> 4096 columns per core). Full-BT tiles everywhere.

Engine assignment for the selective scan (the bottleneck):
  ACT   : dA = exp(A[d,n] * dt)            (per-partition scale)
  DVE   : dBu = (dt*u) . B_n ; h = tensor_tensor_scan(dA, dBu) ; hc = h . C_n
  PE    : y = diag(D) @ u + sum_n I @ hc_n (PSUM accumulation, skip term first)
GpSimd does NO compute: running Pool tensor_tensor concurrently with DVE
tensor_tensor degrades BOTH ~2-4x (SBUF port contention) — measured, twice.
DVE tensor_tensor in packed bf16 SBUF hits the 2x mode (~0.56 ns/col); the
scan runs at ~2.1 ns/col + ~2us fixed, which is the hard floor here.
Batch independence inside one scan op is enforced by poisoning dt at each
batch's first column (dt=1e9 -> dA=exp(-big)=0 -> exact state reset).

B/C rows are spilled to DRAM once per layer and partition-broadcast to
[128, BT] tiles per (d, n) (SP queue for B, gpsimd SWDGE for C).
Weights are loaded via gpsimd casting DMAs (f32->bf16) and PE transposes.
Big transients share one 9-deep rotating slot tag ("W"); slot-reuse WAR
edges are safe because every W tile's readers are emitted within 9
subsequent W allocations.
"""
import sys

if "/opt/trn_rl_repo" not in sys.path:
    sys.path.insert(0, "/opt/trn_rl_repo")

from contextlib import ExitStack

import numpy as np

import concourse.bacc as bacc
import concourse.tile as tile
import concourse.mybir as mybir
from concourse.bass_utils import run_bass_kernel_spmd

f32 = mybir.dt.float32
bf16 = mybir.dt.bfloat16
AF = mybir.ActivationFunctionType
ALU = mybir.AluOpType
AX = mybir.AxisListType

# model dims
B, L, VD, ID = 64, 512, 64, 32
H, DI, DS, DC, DR, NL = 256, 512, 16, 4, 16, 2
NCORES = 8
BS = B // NCORES          # batches per core
BT = BS * L               # free columns per core (4096)
LP = L + DC - 1           # padded per-batch length for conv (515)
PBT = BS * LP             # 4120
HB = H // 128             # 2
DB = DI // 128            # 4
POISON = 1.0e9


WEIGHT_NAMES = [
    "vent_in_w", "vent_in_b", "vent_ln_w", "vent_ln_b",
    "m_in_w", "m_conv_w", "m_conv_b", "m_xproj_w", "m_dt_w", "m_dt_b",
    "m_Alog", "m_D", "m_out_w", "m_ln_w", "m_ln_b",
    "pool_w", "pool_b", "img_w1", "img_b1", "img_w2", "img_b2",
    "head_w1", "head_b1", "head_w2", "head_b2",
]


def _build():
    nc = bacc.Bacc("TRN2", target_bir_lowering=False, debug=False)

    # ---- DRAM I/O ----
    xv_d = nc.dram_tensor("xv", [BS, L, VD], f32, kind="ExternalInput")
    xi_d = nc.dram_tensor("xi", [BS, ID], f32, kind="ExternalInput")
    wd = {}
    for name, shape in [
        ("vent_in_w", [H, VD]), ("vent_in_b", [H]), ("vent_ln_w", [H]), ("vent_ln_b", [H]),
        ("m_in_w", [NL, 2 * DI, H]), ("m_conv_w", [NL, DI, DC]), ("m_conv_b", [NL, DI]),
        ("m_xproj_w", [NL, DR + 2 * DS, DI]), ("m_dt_w", [NL, DI, DR]), ("m_dt_b", [NL, DI]),
        ("m_Alog", [NL, DI, DS]), ("m_D", [NL, DI]), ("m_out_w", [NL, H, DI]),
        ("m_ln_w", [NL, H]), ("m_ln_b", [NL, H]),
        ("pool_w", [1, H]), ("pool_b", [1]),
        ("img_w1", [H, ID]), ("img_b1", [H]), ("img_w2", [H, H]), ("img_b2", [H]),
        ("head_w1", [H, 3 * H]), ("head_b1", [H]), ("head_w2", [1, H]), ("head_b2", [1]),
    ]:
        wd[name] = nc.dram_tensor(name, shape, f32, kind="ExternalInput")
    out_d = nc.dram_tensor("out", [1, BS], f32, kind="ExternalOutput")

    # DRAM scratch
    bc_sp = nc.dram_tensor("bc_sp", [2 * DS, BT], bf16)   # B rows 0:16, C rows 16:32
    z_sp = nc.dram_tensor("z_sp", [DI, BT], bf16)         # silu(z) spill
    dt_sp = nc.dram_tensor("dt_sp", [2, 128, BT], bf16)   # dt spill for d=2,3
    dtu_sp = nc.dram_tensor("dtu_sp", [2, 128, BT], bf16)
    st_sp = nc.dram_tensor("st_sp", [2, BT], bf16)        # LN mu/inv bf16 rows
    ex_sp = nc.dram_tensor("ex_sp", [1, BT], bf16)        # attn-pool exp row
    rs_sp = nc.dram_tensor("rs_sp", [1, BS], f32)         # attn-pool 1/sum

    with tile.TileContext(nc) as tc, ExitStack() as ctx:
        wpool = ctx.enter_context(tc.tile_pool(name="wpool", bufs=1))
        ap = ctx.enter_context(tc.tile_pool(name="ap", bufs=2))

        WBUFS = 9

        def wtile(name):
            """Big rotating transient slot [128, <=4120]."""
            return ap.tile([128, BT], bf16, tag="W", bufs=WBUFS, name=name)

        # ---------------- constants ----------------
        ident = wpool.tile([128, 128], bf16, name="ident")
        nc.vector.memset(ident[:], 1.0)
        nc.gpsimd.affine_select(ident[:], ident[:], pattern=[[-1, 128]], base=0,
                                channel_multiplier=1, compare_op=ALU.is_equal, fill=0.0)
        ones_col = wpool.tile([128, 1], bf16, name="ones_col")
        nc.vector.memset(ones_col[:], 1.0)
        eps_col = wpool.tile([128, 1], f32, name="eps_col")
        nc.vector.memset(eps_col[:], 1e-5)

        # ---------------- input DMAs first (own the SP queue head) ----------
        xiT = ap.tile([ID, BS], f32, tag="xiT", name="xiT")
        nc.sync.dma_start(xiT[:], xi_d.ap().rearrange("b f -> f b"))

        xv16 = ap.tile([128, BS * L * VD // 128], bf16, tag="W", bufs=WBUFS, name="xv16")
        xv_3d = xv_d.ap().rearrange("b l v -> (b l) v").rearrange("(c p) v -> p c v", p=128)
        nc.gpsimd.dma_start(
            xv16[:].rearrange("p (c v) -> p c v", v=VD), xv_3d)

        # ---------------- weight preprocessing ----------------
        ld_ctx = ExitStack()
        ldp = ld_ctx.enter_context(tc.tile_pool(name="ldp", bufs=3))
        ldps = ld_ctx.enter_context(tc.tile_pool(name="ldps", bufs=2, space="PSUM"))

        def load_cols(src_ap, n, name):
            """1-D DRAM vector [n] -> list of [128,1] f32 col tiles."""
            cols = []
            for blk in range((n + 127) // 128):
                m = min(128, n - blk * 128)
                t = wpool.tile([m, 1], f32, name=f"{name}_c{blk}")
                nc.sync.dma_start(t[:, 0:1],
                                  src_ap[blk * 128: blk * 128 + m].rearrange("(a b) -> a b", b=1))
                cols.append(t)
            return cols

        def load_T(src_ap, R, C, name):
            """DRAM [R, C] f32 -> transposed bf16 SBUF tiles (list over C-blocks of [cm, R]).

            Casting f32->bf16 happens inside the gpsimd software-DGE DMA."""
            nrb = (R + 127) // 128
            ncb = (C + 127) // 128
            outs = []
            for cb in range(ncb):
                cm = min(128, C - cb * 128)
                t = wpool.tile([cm, R], bf16, name=f"{name}_T{cb}")
                outs.append(t)
            for rb in range(nrb):
                rm = min(128, R - rb * 128)
                nat16 = ldp.tile([rm, C], bf16, tag="ld16", name=f"{name}_m{rb}")
                nc.gpsimd.dma_start(nat16[:], src_ap[rb * 128: rb * 128 + rm, :])
                for cb in range(ncb):
                    cm = min(128, C - cb * 128)
                    tp = ldps.tile([cm, rm], bf16, tag="ldT", name=f"{name}_p{rb}_{cb}")
                    nc.tensor.transpose(tp[:], nat16[:, cb * 128: cb * 128 + cm],
                                        ident[0:rm, 0:rm])
                    nc.vector.tensor_copy(outs[cb][:, rb * 128: rb * 128 + rm], tp[:])
            return outs

        ventT = load_T(wd["vent_in_w"].ap(), H, VD, "ventT")          # 1 x [64, 256]
        vent_b = load_cols(wd["vent_in_b"].ap(), H, "vent_b")
        vlnw = load_cols(wd["vent_ln_w"].ap(), H, "vlnw")
        vlnb = load_cols(wd["vent_ln_b"].ap(), H, "vlnb")

        # ---------------- layernorm over H (partition dim) ----------------
        def layernorm(xo, w_cols, b_cols, tag):
            """xo: HB bf16 [128, BT] tiles (pre-norm) -> normalized tiles (tag 'x')."""
            sq = []
            for hb in range(HB):
                sqt = wtile(f"sq_{tag}_{hb}")
                nc.vector.tensor_tensor(sqt[:], xo[hb][:], xo[hb][:], ALU.mult)
                sq.append(sqt)
            mu8 = ap.tile([BS, 512], f32, tag="ln8", bufs=4, name=f"mu8_{tag}")
            msq8 = ap.tile([BS, 512], f32, tag="ln8", bufs=4, name=f"msq8_{tag}")
            with tc.tile_pool(name=f"lnps_{tag}", bufs=2, space="PSUM") as lps:
                for s in range(BS):
                    sl = slice(s * 512, (s + 1) * 512)
                    ps_x = lps.tile([1, 512], f32, tag="lnst1", name=f"sx_{tag}_{s}")
                    for hb in range(HB):
                        nc.tensor.matmul(ps_x[:], ones_col[:], xo[hb][:, sl],
                                         start=(hb == 0), stop=(hb == HB - 1))
                    sxs = ap.tile([1, 512], f32, tag="lnsl", bufs=2, name=f"sxs_{tag}_{s}")
                    nc.scalar.activation(sxs[:], ps_x[:], AF.Copy, scale=1.0 / H)
                    nc.sync.dma_start(mu8[s:s + 1, :], sxs[:])
                    ps_q = lps.tile([1, 512], f32, tag="lnst2", name=f"sq_{tag}_{s}")
                    for hb in range(HB):
                        nc.tensor.matmul(ps_q[:], ones_col[:], sq[hb][:, sl],
                                         start=(hb == 0), stop=(hb == HB - 1))
                    sqs2 = ap.tile([1, 512], f32, tag="lnsl", bufs=2, name=f"sqs_{tag}_{s}")
                    nc.scalar.activation(sqs2[:], ps_q[:], AF.Copy, scale=1.0 / H)
                    nc.sync.dma_start(msq8[s:s + 1, :], sqs2[:])
            var8 = ap.tile([BS, 512], f32, tag="ln8", bufs=4, name=f"var8_{tag}")
            nc.vector.tensor_tensor(var8[:], mu8[:], mu8[:], ALU.mult)
            nc.vector.tensor_tensor(var8[:], msq8[:], var8[:], ALU.subtract)
            sd8 = ap.tile([BS, 512], f32, tag="ln8", bufs=4, name=f"sd8_{tag}")
            nc.scalar.activation(sd8[:], var8[:], AF.Sqrt, bias=eps_col[0:BS, 0:1])
            inv8 = ap.tile([BS, 512], f32, tag="ln8", bufs=4, name=f"inv8_{tag}")
            nc.vector.reciprocal(inv8[:], sd8[:])
            mu16 = ap.tile([BS, 512], bf16, tag="ln16", bufs=2, name=f"mu16_{tag}")
            nc.vector.tensor_copy(mu16[:], mu8[:])
            inv16 = ap.tile([BS, 512], bf16, tag="ln16", bufs=2, name=f"inv16_{tag}")
            nc.vector.tensor_copy(inv16[:], inv8[:])
            nc.sync.dma_start(st_sp.ap()[0, :].rearrange("(b t) -> b t", b=BS), mu16[:])
            nc.sync.dma_start(st_sp.ap()[1, :].rearrange("(b t) -> b t", b=BS), inv16[:])
            mu_rep = wtile(f"murep_{tag}")
            nc.sync.dma_start(mu_rep[:], st_sp.ap()[0, :].partition_broadcast(128))
            inv_rep = wtile(f"invrep_{tag}")
            nc.scalar.dma_start(inv_rep[:], st_sp.ap()[1, :].partition_broadcast(128))
            xcs = []
            for hb in range(HB):
                xc = wtile(f"xc_{tag}_{hb}")
                nc.vector.tensor_tensor(xc[:], xo[hb][:], mu_rep[:], ALU.subtract)
                xcs.append(xc)
            x_out = []
            for hb in range(HB):
                xn = wtile(f"xn_{tag}_{hb}")
                nc.vector.tensor_tensor(xn[:], xcs[hb][:], inv_rep[:], ALU.mult)
                xt = ap.tile([128, BT], bf16, tag="x", bufs=2, name=f"x_{tag}_{hb}")
                nc.scalar.activation(xt[:], xn[:], AF.Identity,
                                     scale=w_cols[hb][:, 0:1], bias=b_cols[hb][:, 0:1])
                x_out.append(xt)
            return x_out

        # ---------------- vent input projection ----------------
        xvT = wtile("xvT")  # [64, BT] on first 64 partitions
        xo0 = []
        with tc.tile_pool(name="xvps", bufs=3, space="PSUM") as xps, \
             tc.tile_pool(name="ventps", bufs=3, space="PSUM") as vps:
            for blk in range(BT // 128):
                tp = xps.tile([VD, 128], bf16, tag="xvT", name=f"xvp{blk}")
                nc.tensor.transpose(tp[:], xv16[:, blk * VD:(blk + 1) * VD], ident[:])
                nc.vector.tensor_copy(xvT[0:VD, blk * 128:(blk + 1) * 128], tp[:])
            for hb in range(HB):
                xo_t = wtile(f"vxo{hb}")
                for s in range(BS):
                    sl = slice(s * 512, (s + 1) * 512)
                    ps = vps.tile([128, 512], f32, tag="pj", name=f"vps{hb}_{s}")
                    nc.tensor.matmul(ps[:], ventT[0][:, hb * 128:(hb + 1) * 128],
                                     xvT[0:VD, sl], start=True, stop=True)
                    nc.scalar.activation(xo_t[:, sl], ps[:], AF.Identity,
                                         bias=vent_b[hb][:, 0:1])
                xo0.append(xo_t)
        x = layernorm(xo0, vlnw, vlnb, "vent")

        inwT, xpwT, dtwT, outwT = [], [], [], []
        conv_w, conv_b, dt_b, A_t, D_t, lnw, lnb = [], [], [], [], [], [], []
        for l in range(NL):
            inwT.append(load_T(wd["m_in_w"].ap()[l], 2 * DI, H, f"inwT{l}"))
            xpwT.append(load_T(wd["m_xproj_w"].ap()[l], DR + 2 * DS, DI, f"xpwT{l}"))
            dtwT.append(load_T(wd["m_dt_w"].ap()[l], DI, DR, f"dtwT{l}"))
            outwT.append(load_T(wd["m_out_w"].ap()[l], H, DI, f"outwT{l}"))
            cwl, al = [], []
            for d in range(DB):
                sl = slice(d * 128, (d + 1) * 128)
                cw = wpool.tile([128, DC], f32, name=f"cw{l}_{d}")
                nc.sync.dma_start(cw[:], wd["m_conv_w"].ap()[l, sl, :])
                cwl.append(cw)
                alog = ldp.tile([128, DS], f32, tag="alog", name=f"alog{l}_{d}")
                nc.sync.dma_start(alog[:], wd["m_Alog"].ap()[l, sl, :])
                a = wpool.tile([128, DS], f32, name=f"A{l}_{d}")
                nc.scalar.activation(a[:], alog[:], AF.Exp)
                nc.vector.tensor_scalar_mul(a[:], a[:], -1.0)
                al.append(a)
            conv_w.append(cwl)
            conv_b.append(load_cols(wd["m_conv_b"].ap()[l], DI, f"cb{l}"))
            dt_b.append(load_cols(wd["m_dt_b"].ap()[l], DI, f"dtb{l}"))
            Dcols = load_cols(wd["m_D"].ap()[l], DI, f"D{l}")
            dgl = []
            for d in range(DB):
                dg = wpool.tile([128, 128], bf16, name=f"dg{l}_{d}")
                nc.vector.tensor_scalar_mul(dg[:], ident[:], Dcols[d][:, 0:1])
                dgl.append(dg)
            D_t.append(dgl)
            A_t.append(al)
            lnw.append(load_cols(wd["m_ln_w"].ap()[l], H, f"lnw{l}"))
            lnb.append(load_cols(wd["m_ln_b"].ap()[l], H, f"lnb{l}"))
        ld_ctx.close()
        # ---------------- mamba layers ----------------
        for l in range(NL):
            if l == 1:
                # head/img/pool weights + image branch: emitted at the layer
                # boundary so their PE/DMA work fills the F/LN gap
                ld2 = ExitStack()
                ldp = ld2.enter_context(tc.tile_pool(name="ldp2", bufs=3))
                ldps = ld2.enter_context(tc.tile_pool(name="ldps2", bufs=2, space="PSUM"))
                poolT = load_T(wd["pool_w"].ap(), 1, H, "poolT")              # 2 x [128, 1]
                poolb = wpool.tile([1, 1], f32, name="poolb")
                nc.sync.dma_start(poolb[:], wd["pool_b"].ap().rearrange("(a b) -> a b", b=1))
                imgw1T = load_T(wd["img_w1"].ap(), H, ID, "imgw1T")           # 1 x [32, 256]
                imgb1 = load_cols(wd["img_b1"].ap(), H, "imgb1")
                imgw2T = load_T(wd["img_w2"].ap(), H, H, "imgw2T")            # 2 x [128, 256]
                imgb2 = load_cols(wd["img_b2"].ap(), H, "imgb2")
                h1T = load_T(wd["head_w1"].ap(), H, 3 * H, "h1T")             # 6 x [128, 256]
                hb1 = load_cols(wd["head_b1"].ap(), H, "hb1")
                h2T = load_T(wd["head_w2"].ap(), 1, H, "h2T")                 # 2 x [128, 1]
                hb2 = wpool.tile([1, 1], f32, name="hb2")
                nc.sync.dma_start(hb2[:], wd["head_b2"].ap().rearrange("(a b) -> a b", b=1))
                ld2.close()
                # ---------------- image branch (independent of the mamba stack) -----
                xiT16 = ap.tile([ID, BS], bf16, tag="xiT16", name="xiT16")
                nc.vector.tensor_copy(xiT16[:], xiT[:])
                ii2 = []
                with tc.tile_pool(name="Ips", bufs=2, space="PSUM") as ips:
                    ii1 = []
                    for hb in range(HB):
                        ps = ips.tile([128, BS], f32, tag="hp", name=f"i1p{hb}")
                        nc.tensor.matmul(ps[:], imgw1T[0][0:ID, hb * 128:(hb + 1) * 128], xiT16[:],
                                         start=True, stop=True)
                        t = ap.tile([128, BS], bf16, tag="ii1t", name=f"ii1_{hb}")
                        nc.scalar.activation(t[:], ps[:], AF.Relu, bias=imgb1[hb][:, 0:1])
                        ii1.append(t)
                    for hb in range(HB):
                        ps = ips.tile([128, BS], f32, tag="hp", name=f"i2p{hb}")
                        for kb in range(HB):
                            nc.tensor.matmul(ps[:], imgw2T[kb][:, hb * 128:(hb + 1) * 128],
                                             ii1[kb][:], start=(kb == 0), stop=(kb == HB - 1))
                        t = ap.tile([128, BS], bf16, tag="ii2t", name=f"ii2_{hb}")
                        nc.scalar.activation(t[:], ps[:], AF.Relu, bias=imgb2[hb][:, 0:1])
                        ii2.append(t)
            u_t = []
            # ---- phase A (u half) + depthwise causal conv + silu ----
            with tc.tile_pool(name=f"Aps{l}", bufs=3, space="PSUM") as aps:
                for d in range(DB):
                    ur = ap.tile([128, PBT], bf16, tag="W", bufs=WBUFS, name=f"uraw{l}_{d}")
                    for b in range(BS):
                        nc.gpsimd.memset(ur[:, b * LP: b * LP + DC - 1], 0.0)
                    for s in range(BS):
                        sl = slice(s * 512, (s + 1) * 512)
                        ps = aps.tile([128, 512], f32, tag="pj", name=f"aps{l}_{d}_{s}")
                        for kb in range(HB):
                            nc.tensor.matmul(ps[:], inwT[l][kb][:, d * 128:(d + 1) * 128],
                                             x[kb][:, sl], start=(kb == 0), stop=(kb == HB - 1))
                        nc.scalar.activation(ur[:, s * LP + DC - 1:(s + 1) * LP], ps[:],
                                             AF.Copy)
                    # conv: full-width shifted 2D slices (stay inside each
                    # 515-seg). ACT does the per-channel scaled shifts (it has
                    # slack); DVE does packed-2x pairwise adds.
                    CW = PBT - DC + 1
                    sh = []
                    for k in range(DC):
                        st = ap.tile([128, PBT], bf16, tag="W", bufs=WBUFS,
                                     name=f"csh{l}_{d}_{k}")
                        nc.vector.tensor_scalar_mul(st[:, 0:CW], ur[:, k:CW + k],
                                                    conv_w[l][d][:, k:k + 1])
                        sh.append(st)
                    a01 = ap.tile([128, PBT], bf16, tag="W", bufs=WBUFS,
                                  name=f"ca01{l}_{d}")
                    nc.vector.tensor_tensor(a01[:, 0:CW], sh[0][:, 0:CW],
                                            sh[1][:, 0:CW], ALU.add)
                    a23 = ap.tile([128, PBT], bf16, tag="W", bufs=WBUFS,
                                  name=f"ca23{l}_{d}")
                    nc.vector.tensor_tensor(a23[:, 0:CW], sh[2][:, 0:CW],
                                            sh[3][:, 0:CW], ALU.add)
                    acc = ap.tile([128, PBT], bf16, tag="W", bufs=WBUFS,
                                  name=f"cacc{l}_{d}")
                    nc.vector.tensor_tensor(acc[:, 0:CW], a01[:, 0:CW],
                                            a23[:, 0:CW], ALU.add)
                    ut = ap.tile([128, BT], bf16, tag="u", bufs=4, name=f"u{l}_{d}")
                    nc.scalar.activation(
                        ut[:].rearrange("p (b t) -> p b t", b=BS),
                        acc[:].rearrange("p (b t) -> p b t", t=LP)[:, :, 0:512],
                        AF.Silu, bias=conv_b[l][d][:, 0:1])
                    u_t.append(ut)
                    # z quarter for this d: fills the PE gap left by conv
                    mb = 4 + d
                    zt = wtile(f"z{l}_{mb}")
                    for s in range(BS):
                        sl = slice(s * 512, (s + 1) * 512)
                        ps = aps.tile([128, 512], f32, tag="pj", name=f"zps{l}_{mb}_{s}")
                        for kb in range(HB):
                            nc.tensor.matmul(ps[:], inwT[l][kb][:, mb * 128:(mb + 1) * 128],
                                             x[kb][:, sl], start=(kb == 0), stop=(kb == HB - 1))
                        nc.vector.tensor_copy(zt[:, sl], ps[:])
                    nc.scalar.dma_start(z_sp.ap()[(mb - 4) * 128:(mb - 3) * 128, :], zt[:])
            # ---- phase C: xproj -> (B | C) rows first (unblocks the E-phase
            # broadcast pipeline), then the dt_in rows ----
            xdb = ap.tile([64, BT], bf16, tag="xd", bufs=1, name=f"xdb{l}")
            xdt = xdb[0:16, :]
            xbc = xdb[32:64, :]
            with tc.tile_pool(name=f"Cps{l}", bufs=3, space="PSUM") as cps:
                for s in range(BS):
                    sl = slice(s * 512, (s + 1) * 512)
                    ps = cps.tile([16, 512], f32, tag="pdt", name=f"cpd{l}_{s}")
                    for kb in range(DB):
                        nc.tensor.matmul(ps[:], xpwT[l][kb][:, 0:16], u_t[kb][:, sl],
                                         start=(kb == 0), stop=(kb == DB - 1))
                    nc.vector.tensor_copy(xdt[:, sl], ps[:])
                for s in range(BS):
                    sl = slice(s * 512, (s + 1) * 512)
                    ps = cps.tile([32, 512], f32, tag="pbc", name=f"cps{l}_{s}")
                    for kb in range(DB):
                        nc.tensor.matmul(ps[:], xpwT[l][kb][:, 16:48], u_t[kb][:, sl],
                                         start=(kb == 0), stop=(kb == DB - 1))
                    nc.vector.tensor_copy(xbc[:, sl], ps[:])
                nc.sync.dma_start(bc_sp.ap()[:, :], xbc[:, :])

            # ---- phase D: dt_proj -> softplus; dtu; poison; spill d>=2 ----
            dt_res, dtu_res = {}, {}
            with tc.tile_pool(name=f"Dps{l}", bufs=3, space="PSUM") as dps:
                for d in range(DB):
                    et = wtile(f"et{l}_{d}")
                    for s in range(BS):
                        sl = slice(s * 512, (s + 1) * 512)
                        ps = dps.tile([128, 512], f32, tag="pj", name=f"dps{l}_{d}_{s}")
                        nc.tensor.matmul(ps[:], dtwT[l][0][0:DR, d * 128:(d + 1) * 128],
                                         xdt[0:DR, sl], start=True, stop=True)
                        # softplus(x+b) = ln(1 + exp(x+b))
                        nc.scalar.activation(et[:, sl], ps[:], AF.Exp,
                                             bias=dt_b[l][d][:, 0:1])
                    if d < 2:
                        dt_t = ap.tile([128, BT], bf16, tag="dt", bufs=2, name=f"dt{l}_{d}")
                    else:
                        dt_t = wtile(f"dtw{l}_{d}")
                    nc.scalar.activation(dt_t[:], et[:], AF.Ln, bias=1.0)
                    if d < 2:
                        dtu = ap.tile([128, BT], bf16, tag="dtu", bufs=2, name=f"dtu{l}_{d}")
                    else:
                        dtu = wtile(f"dtuw{l}_{d}")
                    nc.vector.tensor_tensor(dtu[:], dt_t[:], u_t[d][:], ALU.mult)
                    for b in range(BS):
                        nc.gpsimd.memset(dt_t[:, b * L: b * L + 1], POISON)
                    if d >= 2:
                        nc.scalar.dma_start(dt_sp.ap()[d - 2], dt_t[:])
                        nc.scalar.dma_start(dtu_sp.ap()[d - 2], dtu[:])
                    else:
                        dt_res[d] = dt_t
                        dtu_res[d] = dtu

            # ---- phase E: selective scan ----
            with tc.tile_pool(name=f"Eps{l}", bufs=1, space="PSUM") as eps_pool:
                for d in range(DB):
                    if d < 2:
                        dtL, dtuL = dt_res[d], dtu_res[d]
                    else:
                        dtL = ap.tile([128, BT], bf16, tag="dt", bufs=2, name=f"dtL{l}_{d}")
                        nc.scalar.dma_start(dtL[:], dt_sp.ap()[d - 2])
                        dtuL = ap.tile([128, BT], bf16, tag="dtu", bufs=2, name=f"dtuL{l}_{d}")
                        nc.scalar.dma_start(dtuL[:], dtu_sp.ap()[d - 2])
                    y_ps = eps_pool.tile([128, BT], f32, tag="y", name=f"yps{l}_{d}")
                    # skip term first: y = diag(D) @ u  (so the accumulation
                    # finishes right after the last state's idents)
                    for si in range(BS):
                        sl = slice(si * 512, (si + 1) * 512)
                        nc.tensor.matmul(y_ps[:, sl], D_t[l][d][:], u_t[d][:, sl],
                                         start=True, stop=False)
                    zs = None
                    for n in range(DS):
                        repB = wtile(f"rb{l}_{d}_{n}")
                        nc.sync.dma_start(repB[:], bc_sp.ap()[n, :].partition_broadcast(128))
                        repC = wtile(f"rc{l}_{d}_{n}")
                        nc.gpsimd.dma_start(repC[:],
                                            bc_sp.ap()[DS + n, :].partition_broadcast(128))
                        dA = wtile(f"dA{l}_{d}_{n}")
                        nc.scalar.activation(dA[:], dtL[:], AF.Exp,
                                             scale=A_t[l][d][:, n:n + 1])
                        dBu = wtile(f"dBu{l}_{d}_{n}")
                        nc.vector.tensor_tensor(dBu[:], dtuL[:], repB[:], ALU.mult)
                        h = wtile(f"h{l}_{d}_{n}")
                        nc.vector.tensor_tensor_scan(h[:], dA[:], dBu[:], 0.0,
                                                     ALU.mult, ALU.add)
                        hc = wtile(f"hc{l}_{d}_{n}")
                        nc.vector.tensor_tensor(hc[:], h[:], repC[:], ALU.mult)
                        for si in range(BS):
                            sl = slice(si * 512, (si + 1) * 512)
                            nc.tensor.matmul(y_ps[:, sl], ident[:], hc[:, sl],
                                             start=False, stop=(n == DS - 1))
                        if n == DS - 2:
                            # prefetch + silu the gate input during the last unit
                            zsr = wtile(f"zsr{l}_{d}")
                            nc.sync.dma_start(zsr[:],
                                              z_sp.ap()[d * 128:(d + 1) * 128, :])
                            zs = wtile(f"zs{l}_{d}")
                            nc.scalar.activation(zs[:], zsr[:], AF.Silu)
                    # gate per 512-chunk, pipelined behind the last ident pass
                    for si in range(BS):
                        sl = slice(si * 512, (si + 1) * 512)
                        nc.vector.tensor_tensor(u_t[d][:, sl], zs[:, sl], y_ps[:, sl],
                                                ALU.mult)

            # ---- phase F: out_proj + layernorm ----
            xo = []
            with tc.tile_pool(name=f"Fps{l}", bufs=3, space="PSUM") as fps:
                for hb in range(HB):
                    xo_t = wtile(f"xo{l}_{hb}")
                    for s in range(BS):
                        sl = slice(s * 512, (s + 1) * 512)
                        ps = fps.tile([128, 512], f32, tag="pj", name=f"fps{l}_{hb}_{s}")
                        for kb in range(DB):
                            nc.tensor.matmul(ps[:], outwT[l][kb][:, hb * 128:(hb + 1) * 128],
                                             u_t[kb][:, sl], start=(kb == 0),
                                             stop=(kb == DB - 1))
                        nc.scalar.activation(xo_t[:, sl], ps[:], AF.Copy)
                    xo.append(xo_t)
            x = layernorm(xo, lnw[l], lnb[l], f"l{l}")

        # ---------------- attention pool over time (softmax, no max-sub:
        # logits are O(0.3) so exp is perfectly stable) ----------------
        ex16 = wtile("ex16")  # [1, BT] used on partition 0
        ex_rep = wtile("ex_rep")
        xa0 = wtile("xa0")
        xa1 = wtile("xa1")
        xas = [xa0, xa1]
        vv = [ap.tile([128, BS], f32, tag="vsm", bufs=2, name=f"vv{hb}")
              for hb in range(HB)]
        with tc.tile_pool(name="Pps", bufs=3, space="PSUM") as pps:
            for s in range(BS):
                sl = slice(s * 512, (s + 1) * 512)
                ps = pps.tile([1, 512], f32, tag="lgst", name=f"pps{s}")
                for hb in range(HB):
                    nc.tensor.matmul(ps[:], poolT[hb][:, 0:1], x[hb][:, sl],
                                     start=(hb == 0), stop=(hb == HB - 1))
                nc.scalar.activation(ex16[0:1, sl], ps[:], AF.Exp, bias=poolb[0:1, 0:1])
                # stream the whole chain per batch: spill, broadcast, weight, reduce
                nc.sync.dma_start(ex_sp.ap()[0:1, sl], ex16[0:1, sl])
                nc.sync.dma_start(ex_rep[:, sl],
                                  ex_sp.ap()[0, sl].partition_broadcast(128))
                for hb in range(HB):
                    nc.vector.tensor_tensor(xas[hb][:, sl], x[hb][:, sl],
                                            ex_rep[:, sl], ALU.mult)
                    nc.vector.tensor_reduce(vv[hb][:, s:s + 1], xas[hb][:, sl],
                                            axis=AX.X, op=ALU.add)
        sm8 = ap.tile([1, BS], f32, tag="smc", name="sm8")
        nc.vector.tensor_reduce(sm8[:], ex16[0:1, :].rearrange("p (b t) -> p b t", b=BS),
                                axis=AX.X, op=ALU.add)
        rs = ap.tile([1, BS], f32, tag="smc", name="rs")
        nc.vector.reciprocal(rs[:], sm8[:])
        nc.sync.dma_start(rs_sp.ap(), rs[:])
        rs_rep = ap.tile([128, BS], f32, tag="rsr", name="rs_rep")
        nc.sync.dma_start(rs_rep[:], rs_sp.ap()[0, :].partition_broadcast(128))
        v_t = []
        for hb in range(HB):
            v16 = ap.tile([128, BS], bf16, tag="vshb", name=f"v16_{hb}")
            nc.vector.tensor_tensor(v16[:], vv[hb][:], rs_rep[:], ALU.mult)
            v_t.append(v16)

        # ---------------- fusion head ----------------
        with tc.tile_pool(name="Hps", bufs=3, space="PSUM") as hps:
            vi = []
            for hb in range(HB):
                t = ap.tile([128, BS], bf16, tag="vit", name=f"vi{hb}")
                nc.vector.tensor_tensor(t[:], v_t[hb][:], ii2[hb][:], ALU.mult)
                vi.append(t)
            f_rhs = [v_t[0], v_t[1], ii2[0], ii2[1], vi[0], vi[1]]
            hh = []
            for mb in range(HB):
                ps = hps.tile([128, BS], f32, tag="hp", name=f"h1p{mb}")
                for kb in range(6):
                    nc.tensor.matmul(ps[:], h1T[kb][:, mb * 128:(mb + 1) * 128],
                                     f_rhs[kb][:], start=(kb == 0), stop=(kb == 5))
                t = ap.tile([128, BS], bf16, tag="hht", name=f"hh{mb}")
                nc.scalar.activation(t[:], ps[:], AF.Relu, bias=hb1[mb][:, 0:1])
                hh.append(t)
            ps = hps.tile([1, BS], f32, tag="hpo", name="outp")
            for kb in range(HB):
                nc.tensor.matmul(ps[:], h2T[kb][:, 0:1], hh[kb][:],
                                 start=(kb == 0), stop=(kb == HB - 1))
            o_sb = ap.tile([1, BS], f32, tag="osb", name="o_sb")
            nc.scalar.activation(o_sb[:], ps[:], AF.Identity, bias=hb2[0:1, 0:1])
        nc.sync.dma_start(out_d.ap(), o_sb[:])

    nc.compile()
    return nc


_NC = None


def _get_nc():
    global _NC
    if _NC is None:
        _NC = _build()
    return _NC


def run(inputs, trace=False):
    nc = _get_nc()
    inputs = {k: np.asarray(v, dtype=np.float32) for k, v in inputs.items()}
    in_maps = []
    for c in range(NCORES):
        m = {name: inputs[name] for name in WEIGHT_NAMES}
        m["xv"] = np.ascontiguousarray(inputs["xv"][c * BS:(c + 1) * BS])
        m["xi"] = np.ascontiguousarray(inputs["xi"][c * BS:(c + 1) * BS])
        in_maps.append(m)
    res = run_bass_kernel_spmd(nc, in_maps, core_ids=list(range(NCORES)), trace=trace)
    out = np.concatenate([np.asarray(res.results[c]["out"]).reshape(BS)
                          for c in range(NCORES)])
    return out.reshape(B, 1).astype(np.float32), res.exec_time_ns


def kernel(**inputs):
    return run(inputs, trace=False)[0]
